# revision 1
# baseline (speedup 1.0000x reference)
"""Complex AttentionPool2d on 8 trn2 NeuronCores, data-parallel over batch.

Contract: kernel(**inputs) takes the FULL inputs from setup_inputs() and
returns the FULL [32, 512] complex64 output.

Math (per batch):
  x = complex(x_real, x_imag).reshape(E, 256); x_cat = [mean(x), x]  # [E, 257]
  x_cat += pos
  q0 = (x_cat[:, 0] @ w_q^T + b_q) / 8          # only query position 0 matters
  k  = x_cat^T @ w_k^T   (k-bias is softmax-invariant -> dropped)
  v  = x_cat^T @ w_v^T   (v-bias folded into final bias on host)
  logits[h, s] = sum_d q0[h*64+d] * k[s, h*64+d]       # complex product
  w = softmax(logits.re) + i*softmax(logits.im)
  attn0 = w @ v                                  # [E]
  y = attn0 @ (w_p @ w_out)^T + b_c              # fused projection, [512]

Sharding: batch 32 -> 4 per core. All big matmuls fp32r (1 cycle/row, N>=256).
"""
import numpy as np

B, E, HW, S = 32, 512, 256, 257
SP = 258            # S padded even for fp32r
NH, HD = 8, 64
OUT = 512
NCORES = 8
BPC = B // NCORES   # batches per core

_cached = {}


def _build():
    import concourse.bacc as bacc
    import concourse.tile as tile
    import concourse.mybir as mybir

    F32 = mybir.dt.float32
    F32R = mybir.dt.float32r
    AX = mybir.AxisListType
    ACTF = mybir.ActivationFunctionType

    nc = bacc.Bacc("TRN2", target_bir_lowering=False, debug=False)

    # ---- DRAM I/O ----
    d_xr = nc.dram_tensor("xr", [BPC, E, HW + 1], F32, kind="ExternalInput")
    d_xi = nc.dram_tensor("xi", [BPC, E, HW + 1], F32, kind="ExternalInput")
    d_posr = nc.dram_tensor("posr", [E, SP], F32, kind="ExternalInput")
    d_posi = nc.dram_tensor("posi", [E, SP], F32, kind="ExternalInput")
    d_wkvr = nc.dram_tensor("wkvr", [E, 2 * E], F32, kind="ExternalInput")
    d_wkvi = nc.dram_tensor("wkvi", [E, 2 * E], F32, kind="ExternalInput")
    d_wqr = nc.dram_tensor("wqr", [E, E], F32, kind="ExternalInput")
    d_wqi = nc.dram_tensor("wqi", [E, E], F32, kind="ExternalInput")
    d_wcr = nc.dram_tensor("wcr", [E, OUT], F32, kind="ExternalInput")
    d_wci = nc.dram_tensor("wci", [E, OUT], F32, kind="ExternalInput")
    d_bqr = nc.dram_tensor("bqr", [128, 4], F32, kind="ExternalInput")
    d_bqi = nc.dram_tensor("bqi", [128, 4], F32, kind="ExternalInput")
    d_bcr = nc.dram_tensor("bcr", [BPC, OUT], F32, kind="ExternalInput")
    d_bci = nc.dram_tensor("bci", [BPC, OUT], F32, kind="ExternalInput")
    d_id = nc.dram_tensor("ident", [128, 128], F32, kind="ExternalInput")
    d_mask = nc.dram_tensor("mask8", [NH, E], F32, kind="ExternalInput")
    d_sel = nc.dram_tensor("sel32", [32, BPC], F32, kind="ExternalInput")
    d_zbd = nc.dram_tensor("zbd", [128, 32], F32, kind="ExternalInput")
    d_yr = nc.dram_tensor("yr", [BPC, OUT], F32, kind="ExternalOutput")
    d_yi = nc.dram_tensor("yi", [BPC, OUT], F32, kind="ExternalOutput")

    with tile.TileContext(nc) as tc:
        with tc.tile_pool(name="consts", bufs=1) as consts, \
             tc.tile_pool(name="vpool", bufs=1) as vpool:
            # ---- persistent weights / constants ----
            wkvr = [consts.tile([128, 2 * E], F32R, name=f"wkvr{e}") for e in range(4)]
            wkvi = [consts.tile([128, 2 * E], F32R, name=f"wkvi{e}") for e in range(4)]
            wkvin = [consts.tile([128, 2 * E], F32R, name=f"wkvin{e}") for e in range(4)]
            wqr = [consts.tile([128, E], F32R, name=f"wqr{e}") for e in range(4)]
            wqi = [consts.tile([128, E], F32R, name=f"wqi{e}") for e in range(4)]
            wcr = [consts.tile([128, OUT], F32R, name=f"wcr{e}") for e in range(4)]
            wci = [consts.tile([128, OUT], F32R, name=f"wci{e}") for e in range(4)]
            posr = [consts.tile([128, SP], F32, name=f"posr{e}") for e in range(4)]
            posi = [consts.tile([128, SP], F32, name=f"posi{e}") for e in range(4)]
            bqr = consts.tile([128, 4], F32)
            bqi = consts.tile([128, 4], F32)
            bqin = consts.tile([128, 4], F32)
            bcr = consts.tile([BPC, OUT], F32)
            bci = consts.tile([BPC, OUT], F32)
            ident = consts.tile([128, 128], F32)
            mask8 = consts.tile([NH, E], F32)
            sel32 = consts.tile([32, BPC], F32)

            # pos first (needed by batch-0 prep); scalar queue so the big
            # sync-queue stream doesn't delay issue
            for e in range(4):
                sl = slice(e * 128, (e + 1) * 128)
                nc.scalar.dma_start(out=posr[e], in_=d_posr.ap()[sl, :])
                nc.scalar.dma_start(out=posi[e], in_=d_posi.ap()[sl, :])

            # v lives until the hv stage; vC and logits live across ktpool exit
            vr = [[vpool.tile([128, OUT], F32R, name=f"vr{b}_{s}")
                   for s in range(2)] for b in range(BPC)]
            vi = [[vpool.tile([128, OUT], F32R, name=f"vi{b}_{s}")
                   for s in range(2)] for b in range(BPC)]
            vCr_sb = vpool.tile([BPC, OUT], F32)
            vCi_sb = vpool.tile([BPC, OUT], F32)
            lg_r = vpool.tile([32, S], F32)
            lg_i = vpool.tile([32, S], F32)

            with tc.tile_pool(name="ktpool", bufs=1) as ktpool:
                kTr = [[ktpool.tile([128, SP], F32R, name=f"kTr{b}_{u}")
                        for u in range(4)] for b in range(BPC)]
                kTi = [[ktpool.tile([128, SP], F32R, name=f"kTi{b}_{u}")
                        for u in range(4)] for b in range(BPC)]
                x0r = [ktpool.tile([128, 4], F32R, name=f"x0r{e}") for e in range(4)]
                x0i = [ktpool.tile([128, 4], F32R, name=f"x0i{e}") for e in range(4)]
                x0in = [ktpool.tile([128, 4], F32R, name=f"x0in{e}") for e in range(4)]
                xlr = [ktpool.tile([128, 4], F32R, name=f"xlr{e}") for e in range(4)]
                xli = [ktpool.tile([128, 4], F32R, name=f"xli{e}") for e in range(4)]

                # ============ PHASE A: x prep + k + v ============
                with tc.tile_pool(name="xpool", bufs=2) as xpool, \
                     tc.tile_pool(name="psA", bufs=2, space="PSUM") as psA:
                    # batch-0 x DMAs land before the weight DMAs in queue order
                    x_pre = {}
                    for e in range(4):
                        sl = slice(e * 128, (e + 1) * 128)
                        xt = xpool.tile([128, SP], F32R, tag=f"xr{e}", name=f"xr_p0_{e}")
                        yt = xpool.tile([128, SP], F32R, tag=f"xi{e}", name=f"xi_p0_{e}")
                        nc.sync.dma_start(out=xt[:, 1:258].bitcast(F32R),
                                          in_=d_xr.ap()[0, sl, :].bitcast(F32R))
                        nc.gpsimd.dma_start(out=yt[:, 1:258].bitcast(F32R),
                                            in_=d_xi.ap()[0, sl, :].bitcast(F32R))
                        for t, pos in ((xt, posr[e]), (yt, posi[e])):
                            with nc.allow_low_precision(reason="f32r holds f32 bits"):
                                nc.vector.reduce_sum(out=t[:, 0:1], in_=t[:, 1:257],
                                                     axis=AX.X)
                            nc.vector.tensor_scalar_mul(t[:, 0:1], t[:, 0:1], 1.0 / HW)
                            nc.vector.tensor_add(t[:], t[:], pos[:])
                        x_pre[e] = (xt, yt)
                    # weights: k-cols first (first k matmul needs them), then v-cols
                    for half in range(2):
                        hs = slice(half * 512, (half + 1) * 512)
                        for e in range(4):
                            sl = slice(e * 128, (e + 1) * 128)
                            nc.sync.dma_start(out=wkvr[e][:, hs].bitcast(F32R),
                                              in_=d_wkvr.ap()[sl, hs].bitcast(F32R))
                            nc.sync.dma_start(out=wkvi[e][:, hs].bitcast(F32R),
                                              in_=d_wkvi.ap()[sl, hs].bitcast(F32R))
                        for e in range(4):
                            with nc.allow_low_precision(reason="f32r holds f32 bits"):
                                nc.vector.tensor_scalar_mul(wkvin[e][:, hs],
                                                            wkvi[e][:, hs], -1.0)
                    # small constants, then late-phase weights
                    nc.sync.dma_start(out=bqr, in_=d_bqr.ap())
                    nc.sync.dma_start(out=bqi, in_=d_bqi.ap())
                    nc.sync.dma_start(out=bcr, in_=d_bcr.ap())
                    nc.sync.dma_start(out=bci, in_=d_bci.ap())
                    nc.sync.dma_start(out=ident, in_=d_id.ap())
                    nc.sync.dma_start(out=mask8, in_=d_mask.ap())
                    nc.sync.dma_start(out=sel32, in_=d_sel.ap())
                    nc.vector.tensor_scalar_mul(bqin, bqi, -1.0)

                    for b in range(BPC):
                        xr_t, xi_t = [], []
                        for e in range(4):
                            sl = slice(e * 128, (e + 1) * 128)
                            if b == 0:
                                xt, yt = x_pre[e]
                            else:
                                xt = xpool.tile([128, SP], F32R, tag=f"xr{e}", name=f"xr_t{b}_{e}")
                                yt = xpool.tile([128, SP], F32R, tag=f"xi{e}", name=f"xi_t{b}_{e}")
                                nc.sync.dma_start(out=xt[:, 1:258].bitcast(F32R),
                                                  in_=d_xr.ap()[b, sl, :].bitcast(F32R))
                                nc.gpsimd.dma_start(out=yt[:, 1:258].bitcast(F32R),
                                                    in_=d_xi.ap()[b, sl, :].bitcast(F32R))
                            xr_t.append(xt)
                            xi_t.append(yt)
                            if b > 0:
                                for t, pos in ((xt, posr[e]), (yt, posi[e])):
                                    with nc.allow_low_precision(reason="f32r holds f32 bits"):
                                        nc.vector.reduce_sum(out=t[:, 0:1],
                                                             in_=t[:, 1:257], axis=AX.X)
                                    nc.vector.tensor_scalar_mul(t[:, 0:1], t[:, 0:1],
                                                                1.0 / HW)
                                    nc.vector.tensor_add(t[:], t[:], pos[:])
                            nc.scalar.copy(x0r[e][:, b:b + 1], xt[:, 0:1])
                            nc.scalar.copy(x0i[e][:, b:b + 1], yt[:, 0:1])
                            nc.scalar.activation(x0in[e][:, b:b + 1], yt[:, 0:1],
                                                 ACTF.Copy, bias=0.0, scale=-1.0)
                            nc.scalar.copy(xlr[e][:, b:b + 1], xt[:, 256:257])
                            nc.scalar.copy(xli[e][:, b:b + 1], yt[:, 256:257])

                        # ---- k^T [f, s]: lhsT = wkv k-cols, rhs = x ----
                        for u in range(4):
                            fs = slice(u * 128, (u + 1) * 128)
                            p1 = psA.tile([128, SP], F32, tag="pk1", name=f"pk1_{b}_{u}")
                            pi = psA.tile([128, SP], F32, tag="pki", name=f"pki_{b}_{u}")
                            for j, (w, x) in enumerate(
                                    [(wkvr[e][:, fs], xr_t[e]) for e in range(4)]
                                    + [(wkvin[e][:, fs], xi_t[e]) for e in range(4)]):
                                nc.tensor.matmul(p1[:], w, x[:], start=(j == 0), stop=(j == 7))
                            for j, (w, x) in enumerate(
                                    [(wkvi[e][:, fs], xr_t[e]) for e in range(4)]
                                    + [(wkvr[e][:, fs], xi_t[e]) for e in range(4)]):
                                nc.tensor.matmul(pi[:], w, x[:], start=(j == 0), stop=(j == 7))
                            nc.vector.tensor_copy(kTr[b][u][:], p1[:])
                            nc.scalar.copy(kTi[b][u][:], pi[:])

                        # ---- v [s, f]: lhsT = x s-block, rhs = wkv v-cols ----
                        for sb in range(2):
                            cs = slice(sb * 128, (sb + 1) * 128)
                            p1 = psA.tile([128, OUT], F32, tag="pv1", name=f"pv1_{b}_{sb}")
                            pi = psA.tile([128, OUT], F32, tag="pvi", name=f"pvi_{b}_{sb}")
                            for j, (x, w) in enumerate(
                                    [(xr_t[e][:, cs], wkvr[e][:, 512:1024]) for e in range(4)]
                                    + [(xi_t[e][:, cs], wkvin[e][:, 512:1024]) for e in range(4)]):
                                nc.tensor.matmul(p1[:], x, w, start=(j == 0), stop=(j == 7))
                            for j, (x, w) in enumerate(
                                    [(xr_t[e][:, cs], wkvi[e][:, 512:1024]) for e in range(4)]
                                    + [(xi_t[e][:, cs], wkvr[e][:, 512:1024]) for e in range(4)]):
                                nc.tensor.matmul(pi[:], x, w, start=(j == 0), stop=(j == 7))
                            nc.vector.tensor_copy(vr[b][sb][:], p1[:])
                            nc.scalar.copy(vi[b][sb][:], pi[:])

                    # late-phase weights: emitted after all x DMAs so they
                    # don't delay the phase-A stream
                    for e in range(4):
                        sl = slice(e * 128, (e + 1) * 128)
                        nc.gpsimd.dma_start(out=wqr[e].bitcast(F32R),
                                            in_=d_wqr.ap()[sl, :].bitcast(F32R))
                        nc.gpsimd.dma_start(out=wqi[e].bitcast(F32R),
                                            in_=d_wqi.ap()[sl, :].bitcast(F32R))
                    for e in range(4):
                        sl = slice(e * 128, (e + 1) * 128)
                        nc.gpsimd.dma_start(out=wcr[e].bitcast(F32R),
                                            in_=d_wcr.ap()[sl, :].bitcast(F32R))
                        nc.gpsimd.dma_start(out=wci[e].bitcast(F32R),
                                            in_=d_wci.ap()[sl, :].bitcast(F32R))

                # ============ PHASE B1: q0 -> bd, vC, logits ============
                with tc.tile_pool(name="miscB1", bufs=1) as mb1:
                    bd_r = mb1.tile([128, 32], F32R)
                    bd_i = mb1.tile([128, 32], F32R)
                    bd_in = mb1.tile([128, 32], F32R)
                    nc.gpsimd.dma_start(out=bd_r[:].bitcast(F32R),
                                        in_=d_zbd.ap()[:].bitcast(F32R))
                    nc.gpsimd.dma_start(out=bd_i[:].bitcast(F32R),
                                        in_=d_zbd.ap()[:].bitcast(F32R))
                    nc.gpsimd.dma_start(out=bd_in[:].bitcast(F32R),
                                        in_=d_zbd.ap()[:].bitcast(F32R))
                    q0r_sb = mb1.tile([BPC, E], F32)
                    q0i_sb = mb1.tile([BPC, E], F32)

                    with tc.tile_pool(name="psB1", bufs=1, space="PSUM") as psB1:
                        # ---- q0 [4b, 512f]: lhsT = x0, rhs = wq^T ----
                        pqr = psB1.tile([BPC, E], F32, tag="pqr")
                        pqi = psB1.tile([BPC, E], F32, tag="pqi")
                        for j, (x, w) in enumerate(
                                [(x0r[e][:], wqr[e][:]) for e in range(4)]
                                + [(x0in[e][:], wqi[e][:]) for e in range(4)]):
                            nc.tensor.matmul(pqr[:], x, w, start=(j == 0), stop=(j == 7))
                        for j, (x, w) in enumerate(
                                [(x0r[e][:], wqi[e][:]) for e in range(4)]
                                + [(x0i[e][:], wqr[e][:]) for e in range(4)]):
                            nc.tensor.matmul(pqi[:], x, w, start=(j == 0), stop=(j == 7))
                        nc.scalar.copy(q0r_sb[:], pqr[:])
                        nc.scalar.copy(q0i_sb[:], pqi[:])

                        # ---- vC: token-256 v row for all batches ----
                        p1 = psB1.tile([BPC, OUT], F32, tag="pc1")
                        pi = psB1.tile([BPC, OUT], F32, tag="pci")
                        for j, (x, w) in enumerate(
                                [(xlr[e][:], wkvr[e][:, 512:1024]) for e in range(4)]
                                + [(xli[e][:], wkvin[e][:, 512:1024]) for e in range(4)]):
                            nc.tensor.matmul(p1[:], x, w, start=(j == 0), stop=(j == 7))
                        for j, (x, w) in enumerate(
                                [(xlr[e][:], wkvi[e][:, 512:1024]) for e in range(4)]
                                + [(xli[e][:], wkvr[e][:, 512:1024]) for e in range(4)]):
                            nc.tensor.matmul(pi[:], x, w, start=(j == 0), stop=(j == 7))
                        nc.scalar.copy(vCr_sb[:], p1[:])
                        nc.scalar.copy(vCi_sb[:], pi[:])

                        # ---- transpose q0 -> bd block-diag [128, 32] ----
                        for u in range(4):
                            fs = slice(u * 128, (u + 1) * 128)
                            ptr = psB1.tile([128, 4], F32, tag="ptq", bufs=2, name=f"ptq{u}")
                            pti = psB1.tile([128, 4], F32, tag="ptj", bufs=2, name=f"ptj{u}")
                            nc.tensor.transpose(ptr[:], q0r_sb[:, fs], ident[0:BPC, 0:BPC])
                            nc.tensor.transpose(pti[:], q0i_sb[:, fs], ident[0:BPC, 0:BPC])
                            for p in range(2):
                                rows = slice(p * 64, (p + 1) * 64)
                                cols = slice(2 * u + p, 32, 8)
                                nc.scalar.activation(bd_r[rows, cols], ptr[rows, :],
                                                     ACTF.Identity,
                                                     bias=bqr[rows, u:u + 1], scale=1.0)
                                nc.scalar.activation(bd_i[rows, cols], pti[rows, :],
                                                     ACTF.Identity,
                                                     bias=bqi[rows, u:u + 1], scale=1.0)
                                nc.scalar.activation(bd_in[rows, cols], pti[rows, :],
                                                     ACTF.Identity,
                                                     bias=bqin[rows, u:u + 1], scale=-1.0)

                    # ---- logits [32, S] (row = b*8 + p*4 + u) ----
                    # two passes (all-real then all-imag) so the real softmax
                    # overlaps the imag logits matmuls on PE
                    with tc.tile_pool(name="psB2", bufs=3, space="PSUM") as psB2:
                        for b in range(BPC):
                            bo_r = mb1.tile([2, 4, SP], F32, tag="bor", bufs=2,
                                            name=f"bo_r{b}")
                            for u in range(4):
                                c0 = b * 8 + 2 * u
                                pr = psB2.tile([2, SP], F32, tag="plr", name=f"plr{b}_{u}")
                                nc.tensor.matmul(pr[:], bd_r[:, c0:c0 + 2], kTr[b][u][:],
                                                 start=True, stop=False)
                                nc.tensor.matmul(pr[:], bd_in[:, c0:c0 + 2], kTi[b][u][:],
                                                 start=False, stop=True)
                                nc.scalar.copy(bo_r[:, u, :], pr[:])
                            nc.sync.dma_start(out=lg_r[b * 8:b * 8 + 8, :],
                                              in_=bo_r[0:2, :, 0:S])
                        for b in range(BPC):
                            bo_i = mb1.tile([2, 4, SP], F32, tag="boi", bufs=2,
                                            name=f"bo_i{b}")
                            for u in range(4):
                                c0 = b * 8 + 2 * u
                                pq = psB2.tile([2, SP], F32, tag="pli", name=f"pli{b}_{u}")
                                nc.tensor.matmul(pq[:], bd_r[:, c0:c0 + 2], kTi[b][u][:],
                                                 start=True, stop=False)
                                nc.tensor.matmul(pq[:], bd_i[:, c0:c0 + 2], kTr[b][u][:],
                                                 start=False, stop=True)
                                nc.vector.tensor_copy(bo_i[:, u, :], pq[:])
                            nc.sync.dma_start(out=lg_i[b * 8:b * 8 + 8, :],
                                              in_=bo_i[0:2, :, 0:S])

            # ============ PHASE B2: softmax, wT, hv, extract, y ============
            with tc.tile_pool(name="miscB2", bufs=1) as mb:
                # vC2[p, b, :]: rows (re, im); vC2s rows (im, re)
                vC2 = mb.tile([2, BPC, OUT], F32R)
                vC2s = mb.tile([2, BPC, OUT], F32R)
                nc.sync.dma_start(out=vC2[0:1, :, :].bitcast(F32R),
                                    in_=vCr_sb[:].bitcast(F32R))
                nc.sync.dma_start(out=vC2[1:2, :, :].bitcast(F32R),
                                    in_=vCi_sb[:].bitcast(F32R))
                nc.sync.dma_start(out=vC2s[0:1, :, :].bitcast(F32R),
                                    in_=vCi_sb[:].bitcast(F32R))
                nc.sync.dma_start(out=vC2s[1:2, :, :].bitcast(F32R),
                                    in_=vCr_sb[:].bitcast(F32R))
                w_ri = mb.tile([32, 2, S], F32)
                w_r = w_ri[:, 0, :]
                w_i = w_ri[:, 1, :]
                for lg, w in ((lg_r, w_r), (lg_i, w_i)):
                    # logits are O(+-8): exp is safe in f32 without max-shift,
                    # and skipping it shortens the serial chain by two hops
                    sm = mb.tile([32, 1], F32, tag="ssm", name=f"sm_{w.name}")
                    rs = mb.tile([32, 1], F32, tag="srs", name=f"rs_{w.name}")
                    nc.scalar.activation(w, lg[:], ACTF.Exp,
                                         bias=0.0, scale=1.0, accum_out=sm[:])
                    nc.vector.reciprocal(rs[:], sm[:])
                    nc.vector.tensor_scalar_mul(w, w, rs[:])

                # ---- transpose w -> wT [S-part, 32] + stacked row-256 tiles ----
                wTr = [mb.tile([128, 32], F32R, name=f"wTr{a}") for a in range(2)]
                wTi = [mb.tile([128, 32], F32R, name=f"wTi{a}") for a in range(2)]
                wTin = [mb.tile([128, 32], F32R, name=f"wTin{a}") for a in range(2)]
                wtc_a = mb.tile([2, 32], F32R)   # rows: wTr_c, -wTi_c
                wtc_b = mb.tile([2, 32], F32R)   # rows: wTr_c, wTi_c
                with tc.tile_pool(name="psB3", bufs=2, space="PSUM") as psB3:
                    for a in range(2):
                        cs = slice(a * 128, (a + 1) * 128)
                        ptr = psB3.tile([128, 32], F32, tag="ptr", name=f"ptr{a}")
                        pti = psB3.tile([128, 32], F32, tag="pti", name=f"pti{a}")
                        nc.tensor.transpose(ptr[:], w_ri[:, 0, cs], ident[0:32, 0:32])
                        nc.tensor.transpose(pti[:], w_ri[:, 1, cs], ident[0:32, 0:32])
                        nc.scalar.copy(wTr[a][:], ptr[:])
                        nc.scalar.copy(wTi[a][:], pti[:])
                        nc.scalar.activation(wTin[a][:], pti[:], ACTF.Copy,
                                             bias=0.0, scale=-1.0)
                    # row-256 of both parts in one [32, 2] -> [2, 32] transpose
                    ptc = psB3.tile([2, 32], F32, tag="ptc")
                    nc.tensor.transpose(ptc[:], w_ri[:, :, 256], ident[0:32, 0:32])
                    wtc_neg = mb.tile([2, 32], F32R)
                    nc.scalar.copy(wtc_b[:], ptc[:])
                    nc.scalar.activation(wtc_neg[:], ptc[:], ACTF.Copy,
                                         bias=0.0, scale=-1.0)
                    # wtc_a rows (re, -im): row copies via DMA (no partition-
                    # alignment restriction there)
                    nc.sync.dma_start(out=wtc_a[0:1, :].bitcast(F32R),
                                      in_=wtc_b[0:1, :].bitcast(F32R))
                    nc.sync.dma_start(out=wtc_a[1:2, :].bitcast(F32R),
                                      in_=wtc_neg[1:2, :].bitcast(F32R))

                # ---- hv: per batch [8, 512]; assemble hvm_all [32, 512] ----
                hvm_r = [mb.tile([NH, OUT], F32, name=f"hvm_r{b}") for b in range(BPC)]
                hvm_i = [mb.tile([NH, OUT], F32, name=f"hvm_i{b}") for b in range(BPC)]
                hvm_all_r = mb.tile([32, OUT], F32)
                hvm_all_i = mb.tile([32, OUT], F32)
                with tc.tile_pool(name="psB4", bufs=2, space="PSUM") as psB4:
                    for b in range(BPC):
                        cols = slice(b * 8, b * 8 + 8)
                        ph_r = psB4.tile([NH, OUT], F32, tag="phr", name=f"phr{b}")
                        ph_i = psB4.tile([NH, OUT], F32, tag="phi", name=f"phi{b}")
                        mm = nc.tensor.matmul
                        mm(ph_r[:], wTr[0][:, cols], vr[b][0][:], start=True, stop=False)
                        mm(ph_r[:], wTr[1][:, cols], vr[b][1][:], start=False, stop=False)
                        mm(ph_r[:], wTin[0][:, cols], vi[b][0][:], start=False, stop=False)
                        mm(ph_r[:], wTin[1][:, cols], vi[b][1][:], start=False, stop=False)
                        mm(ph_r[:], wtc_a[:, cols], vC2[:, b, :], start=False, stop=True)
                        mm(ph_i[:], wTi[0][:, cols], vr[b][0][:], start=True, stop=False)
                        mm(ph_i[:], wTi[1][:, cols], vr[b][1][:], start=False, stop=False)
                        mm(ph_i[:], wTr[0][:, cols], vi[b][0][:], start=False, stop=False)
                        mm(ph_i[:], wTr[1][:, cols], vi[b][1][:], start=False, stop=False)
                        mm(ph_i[:], wtc_b[:, cols], vC2s[:, b, :], start=False, stop=True)
                        nc.vector.tensor_mul(hvm_r[b][:], ph_r[:], mask8[:])
                        nc.vector.tensor_mul(hvm_i[b][:], ph_i[:], mask8[:])
                        nc.sync.dma_start(out=hvm_all_r[b * 8:b * 8 + 8, :],
                                          in_=hvm_r[b][:])
                        nc.sync.dma_start(out=hvm_all_i[b * 8:b * 8 + 8, :],
                                          in_=hvm_i[b][:])

                # ---- extract attn0^T [128, 4] per f-tile via selection matmul ----
                att_r = [mb.tile([128, 4], F32R, name=f"att_r{u}") for u in range(4)]
                att_i = [mb.tile([128, 4], F32R, name=f"att_i{u}") for u in range(4)]
                att_in = [mb.tile([128, 4], F32R, name=f"att_in{u}") for u in range(4)]
                with tc.tile_pool(name="psB5", bufs=2, space="PSUM") as psB5:
                    for u in range(4):
                        fs = slice(u * 128, (u + 1) * 128)
                        par = psB5.tile([128, 4], F32, tag="par", name=f"par{u}")
                        pai = psB5.tile([128, 4], F32, tag="pai", name=f"pai{u}")
                        nc.tensor.matmul(par[:], hvm_all_r[:, fs], sel32[:],
                                         start=True, stop=True)
                        nc.tensor.matmul(pai[:], hvm_all_i[:, fs], sel32[:],
                                         start=True, stop=True)
                        nc.scalar.copy(att_r[u][:], par[:])
                        nc.scalar.copy(att_i[u][:], pai[:])
                        nc.scalar.activation(att_in[u][:], pai[:], ACTF.Copy,
                                             bias=0.0, scale=-1.0)

                # ---- y = attn0 @ Wc^T + b_c ----
                yr_sb = mb.tile([BPC, OUT], F32)
                yi_sb = mb.tile([BPC, OUT], F32)
                with tc.tile_pool(name="psB6", bufs=1, space="PSUM") as psB6:
                    py_r = psB6.tile([BPC, OUT], F32, tag="pyr")
                    py_i = psB6.tile([BPC, OUT], F32, tag="pyi")
                    for j, u in enumerate(range(4)):
                        nc.tensor.matmul(py_r[:], att_r[u][:], wcr[u][:],
                                         start=(j == 0), stop=False)
                        nc.tensor.matmul(py_r[:], att_in[u][:], wci[u][:],
                                         start=False, stop=(j == 3))
                        nc.tensor.matmul(py_i[:], att_r[u][:], wci[u][:],
                                         start=(j == 0), stop=False)
                        nc.tensor.matmul(py_i[:], att_i[u][:], wcr[u][:],
                                         start=False, stop=(j == 3))
                    nc.vector.tensor_add(yr_sb[:], py_r[:], bcr[:])
                    nc.vector.tensor_add(yi_sb[:], py_i[:], bci[:])
                    nc.sync.dma_start(out=d_yr.ap(), in_=yr_sb[:])
                    nc.sync.dma_start(out=d_yi.ap(), in_=yi_sb[:])

    nc.compile()
    return nc


def _host_prep(inputs):
    """Build per-core in_maps from the full inputs."""
    f32 = np.float32
    xr = np.ascontiguousarray(inputs["x_real"], dtype=f32).reshape(B, E, HW)
    xi = np.ascontiguousarray(inputs["x_imag"], dtype=f32).reshape(B, E, HW)
    pos_r = np.asarray(inputs["pos_r"], dtype=f32)
    pos_i = np.asarray(inputs["pos_i"], dtype=f32)
    w_in_r = np.asarray(inputs["w_in_r"], dtype=f32)
    w_in_i = np.asarray(inputs["w_in_i"], dtype=f32)
    b_in_r = np.asarray(inputs["b_in_r"], dtype=f32)
    b_in_i = np.asarray(inputs["b_in_i"], dtype=f32)
    w_out = np.asarray(inputs["w_out_r"], dtype=f32) + 1j * np.asarray(inputs["w_out_i"], dtype=f32)
    b_out = np.asarray(inputs["b_out_r"], dtype=f32) + 1j * np.asarray(inputs["b_out_i"], dtype=f32)
    w_p = np.asarray(inputs["w_p_r"], dtype=f32) + 1j * np.asarray(inputs["w_p_i"], dtype=f32)
    b_p = np.asarray(inputs["b_p_r"], dtype=f32) + 1j * np.asarray(inputs["b_p_i"], dtype=f32)

    posr = np.zeros((E, SP), f32)
    posi = np.zeros((E, SP), f32)
    posr[:, :S] = pos_r
    posi[:, :S] = pos_i

    wkvr = np.ascontiguousarray(w_in_r[E:3 * E].T)          # [E, 2E]
    wkvi = np.ascontiguousarray(w_in_i[E:3 * E].T)
    qs = f32(1.0 / np.sqrt(HD))
    wqr = np.ascontiguousarray(w_in_r[:E].T * qs)           # [E, E]
    wqi = np.ascontiguousarray(w_in_i[:E].T * qs)
    bq_r = (b_in_r[:E] * qs).reshape(4, 128).T.copy()       # [128, 4]
    bq_i = (b_in_i[:E] * qs).reshape(4, 128).T.copy()

    wc = w_p @ w_out                                        # [OUT, E] complex
    wcr = np.ascontiguousarray(wc.real.T.astype(f32))       # [E, OUT]
    wci = np.ascontiguousarray(wc.imag.T.astype(f32))

    b_v = b_in_r[2 * E:] + 1j * b_in_i[2 * E:]
    b_c = (1 + 1j) * (b_v @ wc.T) + b_out @ w_p.T + b_p     # [OUT] complex
    bcr = np.broadcast_to(b_c.real.astype(f32), (BPC, OUT)).copy()
    bci = np.broadcast_to(b_c.imag.astype(f32), (BPC, OUT)).copy()

    ident = np.eye(128, dtype=f32)
    # hv lhsT column c corresponds to head sigma(c) = [0,2,4,6,1,3,5,7][c]
    # (logits rows are stored p-major: row = b*8 + p*4 + u, head = 2u+p)
    sigma = [0, 2, 4, 6, 1, 3, 5, 7]
    mask8 = np.zeros((NH, E), f32)
    for c in range(NH):
        h = sigma[c]
        mask8[c, h * HD:(h + 1) * HD] = 1.0
    sel32 = np.zeros((32, BPC), f32)
    for b in range(BPC):
        sel32[b * 8:(b + 1) * 8, b] = 1.0

    shared = dict(posr=posr, posi=posi, wkvr=wkvr, wkvi=wkvi, wqr=wqr, wqi=wqi,
                  wcr=wcr, wci=wci, bqr=bq_r, bqi=bq_i, bcr=bcr, bci=bci,
                  ident=ident, mask8=mask8, sel32=sel32,
                  zbd=np.zeros((128, 32), f32))
    xrp = np.zeros((B, E, HW + 1), f32)
    xip = np.zeros((B, E, HW + 1), f32)
    xrp[:, :, :HW] = xr
    xip[:, :, :HW] = xi
    in_maps = []
    for c in range(NCORES):
        m = dict(shared)
        m["xr"] = np.ascontiguousarray(xrp[c * BPC:(c + 1) * BPC])
        m["xi"] = np.ascontiguousarray(xip[c * BPC:(c + 1) * BPC])
        in_maps.append(m)
    return in_maps


def _run(inputs, trace=False, **kw):
    from concourse.bass_utils import run_bass_kernel_spmd
    if "nc" not in _cached:
        _cached["nc"] = _build()
    nc = _cached["nc"]
    in_maps = _host_prep(inputs)
    res = run_bass_kernel_spmd(nc, in_maps, core_ids=list(range(NCORES)),
                               trace=trace, **kw)
    out = np.empty((B, OUT), np.complex64)
    for c in range(NCORES):
        out[c * BPC:(c + 1) * BPC] = (res.results[c]["yr"]
                                      + 1j * res.results[c]["yi"])
    return out, res


def kernel(**inputs) -> np.ndarray:
    out, _ = _run(inputs)
    return out



# revision 13
# speedup vs baseline: 1.3164x; 1.3164x over previous
"""Complex AttentionPool2d on 8 trn2 NeuronCores, data-parallel over batch.

Contract: kernel(**inputs) takes the FULL inputs from setup_inputs() and
returns the FULL [32, 512] complex64 output.

v2 design (vs baseline): all matmuls in bf16 (halves DMA, tolerance is 2e-2),
Karatsuba (3 real matmuls) for the dominant k/v projections, and every
small/serial piece of math moved to the host:
  host: x0 = mean(x)+pos0; q0 = (x0 Wq + b_q)/8; k0 = x0 Wk; v0 = x0 Wv;
        logit[s=0] = q0.k0; block-diag bd tiles from q0; v0 row-pairs;
        y-bias (v-bias fold + out biases) added to the final output on host.
  device (per core, 4 batches as 2 column-packed pairs):
        x_cat[:,1:257] = pixels + pos            # [128e, 512 = 2b x 256s]
        kT[f, (b,s)]   = Wk @ x_cat              # Karatsuba, f-major
        v[(b,st)]      = x_cat^T @ Wv            # Karatsuba, s-major
        logits[8h,256] = bd^T @ kT per batch; col 0 DMA'd from host
        w = softmax(re) + i softmax(im)          # exp straight from PSUM
        attn0 = w^T v (+ w0 x v0 row term)       # per batch [8, 512]
        y = attn0 @ (w_p w_out)^T                # via sel-extracted att^T

Math identities: k-bias dropped (softmax invariant); v-bias exits through
sum(w)=1 as a constant y-offset (host-added); q-bias folded into host q0.
"""
import numpy as np
import ml_dtypes

B, E, HW, S = 32, 512, 256, 257
NH, HD = 8, 64
OUT = 512
NCORES = 8
BPC = B // NCORES   # batches per core
NPAIR = BPC // 2    # column-packed batch pairs
BF16 = ml_dtypes.bfloat16

_cached = {}


def _build():
    import concourse.bacc as bacc
    import concourse.tile as tile
    import concourse.mybir as mybir

    F32 = mybir.dt.float32
    BF = mybir.dt.bfloat16
    ACTF = mybir.ActivationFunctionType

    nc = bacc.Bacc("TRN2", target_bir_lowering=False, debug=False)

    # ---- DRAM I/O ----
    d_xr = nc.dram_tensor("xr", [BPC, E, HW], BF, kind="ExternalInput")
    d_xi = nc.dram_tensor("xi", [BPC, E, HW], BF, kind="ExternalInput")
    d_posr = nc.dram_tensor("posr2", [E, 512], BF, kind="ExternalInput")
    d_posi = nc.dram_tensor("posi2", [E, 512], BF, kind="ExternalInput")
    d_wr = nc.dram_tensor("wr", [E, 2 * E], BF, kind="ExternalInput")
    d_wi = nc.dram_tensor("wi", [E, 2 * E], BF, kind="ExternalInput")
    d_ws = nc.dram_tensor("ws", [E, 2 * E], BF, kind="ExternalInput")
    d_wcr = nc.dram_tensor("wcr", [E, OUT], BF, kind="ExternalInput")
    d_wci = nc.dram_tensor("wci", [E, OUT], BF, kind="ExternalInput")
    d_bdr = nc.dram_tensor("bdr", [E, 32], BF, kind="ExternalInput")
    d_bdi = nc.dram_tensor("bdi", [E, 32], BF, kind="ExternalInput")
    d_bdin = nc.dram_tensor("bdin", [E, 32], BF, kind="ExternalInput")
    d_lg0 = nc.dram_tensor("lg0", [32, 2], F32, kind="ExternalInput")
    d_v02 = nc.dram_tensor("v02", [2, BPC, OUT], BF, kind="ExternalInput")
    d_v02s = nc.dram_tensor("v02s", [2, BPC, OUT], BF, kind="ExternalInput")
    d_id32 = nc.dram_tensor("id32", [32, 32], BF, kind="ExternalInput")
    d_mask = nc.dram_tensor("mask8", [NH, OUT], BF, kind="ExternalInput")
    d_sel = nc.dram_tensor("sel32", [32, BPC], BF, kind="ExternalInput")
    d_yr = nc.dram_tensor("yr", [BPC, OUT], F32, kind="ExternalOutput")
    d_yi = nc.dram_tensor("yi", [BPC, OUT], F32, kind="ExternalOutput")

    KS = slice(0, 512)       # k columns of the packed kv weight
    VS = slice(512, 1024)    # v columns

    with tile.TileContext(nc) as tc, \
         nc.allow_low_precision(reason="bf16 kernel; tolerance is 2e-2"):
        with tc.tile_pool(name="consts", bufs=1) as consts, \
             tc.tile_pool(name="keep", bufs=1) as keep:
            # ---- persistent weights / constants ----
            w_r = [consts.tile([128, 2 * E], BF, name=f"wr{e}") for e in range(4)]
            w_i = [consts.tile([128, 2 * E], BF, name=f"wi{e}") for e in range(4)]
            w_s = [consts.tile([128, 2 * E], BF, name=f"ws{e}") for e in range(4)]
            pos2r = [consts.tile([128, 512], BF, name=f"p2r{e}") for e in range(4)]
            pos2i = [consts.tile([128, 512], BF, name=f"p2i{e}") for e in range(4)]
            wcr = [consts.tile([128, OUT], BF, name=f"wcr{e}") for e in range(4)]
            wci = [consts.tile([128, OUT], BF, name=f"wci{e}") for e in range(4)]
            bd_r = [consts.tile([128, 32], BF, name=f"bdr{u}") for u in range(4)]
            bd_i = [consts.tile([128, 32], BF, name=f"bdi{u}") for u in range(4)]
            bd_in = [consts.tile([128, 32], BF, name=f"bdin{u}") for u in range(4)]
            lg0 = consts.tile([32, 2], F32)
            v02 = consts.tile([2, BPC, OUT], BF)
            v02s = consts.tile([2, BPC, OUT], BF)
            id32 = consts.tile([32, 32], BF)
            mask8 = consts.tile([NH, OUT], BF)
            sel32 = consts.tile([32, BPC], BF)

            # ---- persistent activations ----
            xr_t = [[keep.tile([128, 512], BF, name=f"xr{p}_{e}")
                     for e in range(4)] for p in range(NPAIR)]
            xi_t = [[keep.tile([128, 512], BF, name=f"xi{p}_{e}")
                     for e in range(4)] for p in range(NPAIR)]
            xs_t = [[keep.tile([128, 512], BF, name=f"xs{p}_{e}")
                     for e in range(4)] for p in range(NPAIR)]
            kTr = [[keep.tile([128, 512], BF, name=f"kTr{p}_{u}")
                    for u in range(4)] for p in range(NPAIR)]
            kTi = [[keep.tile([128, 512], BF, name=f"kTi{p}_{u}")
                    for u in range(4)] for p in range(NPAIR)]
            vr = [[keep.tile([128, OUT], BF, name=f"vr{b}_{s}")
                   for s in range(2)] for b in range(BPC)]
            vi = [[keep.tile([128, OUT], BF, name=f"vi{b}_{s}")
                   for s in range(2)] for b in range(BPC)]
            w_sm = keep.tile([32, 2, S], BF)        # softmax weights (re|im)
            wexp = [keep.tile([NH, 2, 256], BF, name=f"wexp{b}")
                    for b in range(BPC)]            # per-batch exp staging
            den8 = [keep.tile([NH, 2], F32, name=f"den8_{b}") for b in range(BPC)]
            den = keep.tile([32, 2], F32)           # exp row-sums of cols 1..256
            e0 = keep.tile([32, 2], F32)            # exp of the s=0 logit
            den2 = keep.tile([32, 2], F32)
            rs = keep.tile([32, 2], F32)
            wTr = [keep.tile([128, 32], BF, name=f"wTr{a}") for a in range(2)]
            wTi = [keep.tile([128, 32], BF, name=f"wTi{a}") for a in range(2)]
            wTin = [keep.tile([128, 32], BF, name=f"wTin{a}") for a in range(2)]
            wt0a = keep.tile([2, 32], BF)           # rows (w0r, -w0i)
            wt0b = keep.tile([2, 32], BF)           # rows (w0r, w0i)
            wt0n = keep.tile([2, 32], BF)
            hvm_r = keep.tile([32, OUT], BF)
            hvm_i = keep.tile([32, OUT], BF)
            hvm_rb = [keep.tile([NH, OUT], BF, name=f"hvr{b}") for b in range(BPC)]
            hvm_ib = [keep.tile([NH, OUT], BF, name=f"hvi{b}") for b in range(BPC)]
            att_r = [keep.tile([128, BPC], BF, name=f"atr{u}") for u in range(4)]
            att_i = [keep.tile([128, BPC], BF, name=f"ati{u}") for u in range(4)]
            att_in = [keep.tile([128, BPC], BF, name=f"atn{u}") for u in range(4)]
            s12 = [keep.tile([128, 512], F32, name=f"s12_{j}") for j in range(2)]
            c2s = [keep.tile([128, 512], F32, name=f"c2_{j}") for j in range(2)]
            y_r = keep.tile([BPC, OUT], F32)
            y_i = keep.tile([BPC, OUT], F32)

            # ---- DMA emission ----
            # scalar queue: pos first (x prep needs it), then smalls
            for e in range(4):
                sl = slice(e * 128, (e + 1) * 128)
                nc.scalar.dma_start(out=pos2r[e], in_=d_posr.ap()[sl, :])
                nc.scalar.dma_start(out=pos2i[e], in_=d_posi.ap()[sl, :])
            for u in range(4):
                sl = slice(u * 128, (u + 1) * 128)
                nc.scalar.dma_start(out=bd_r[u], in_=d_bdr.ap()[sl, :])
                nc.scalar.dma_start(out=bd_in[u], in_=d_bdin.ap()[sl, :])
                nc.scalar.dma_start(out=bd_i[u], in_=d_bdi.ap()[sl, :])
            nc.scalar.dma_start(out=lg0, in_=d_lg0.ap())
            nc.scalar.dma_start(out=v02, in_=d_v02.ap())
            nc.scalar.dma_start(out=v02s, in_=d_v02s.ap())
            nc.scalar.dma_start(out=id32, in_=d_id32.ap())
            nc.scalar.dma_start(out=mask8, in_=d_mask.ap())
            nc.scalar.dma_start(out=sel32, in_=d_sel.ap())

            # sync queue: k weights (r, i, s), then v weights, then wc
            for d_w, w_t in ((d_wr, w_r), (d_wi, w_i), (d_ws, w_s)):
                for e in range(4):
                    sl = slice(e * 128, (e + 1) * 128)
                    nc.sync.dma_start(out=w_t[e][:, KS], in_=d_w.ap()[sl, KS])
            for d_w, w_t in ((d_wr, w_r), (d_wi, w_i), (d_ws, w_s)):
                for e in range(4):
                    sl = slice(e * 128, (e + 1) * 128)
                    nc.sync.dma_start(out=w_t[e][:, VS], in_=d_w.ap()[sl, VS])
            for e in range(4):
                sl = slice(e * 128, (e + 1) * 128)
                nc.sync.dma_start(out=wcr[e], in_=d_wcr.ap()[sl, :])
                nc.sync.dma_start(out=wci[e], in_=d_wci.ap()[sl, :])

            # gpsimd queue: x tiles, pair-packed [128e, 2b x 256s]
            for p in range(NPAIR):
                for e in range(4):
                    sl = slice(e * 128, (e + 1) * 128)
                    for h in range(2):
                        cs = slice(h * 256, (h + 1) * 256)
                        nc.gpsimd.dma_start(out=xr_t[p][e][:, cs],
                                            in_=d_xr.ap()[2 * p + h, sl, :])
                        nc.gpsimd.dma_start(out=xi_t[p][e][:, cs],
                                            in_=d_xi.ap()[2 * p + h, sl, :])

            # ---- x prep: add pos, build Karatsuba sum ----
            for p in range(NPAIR):
                for e in range(4):
                    eng = nc.vector if e % 2 == 0 else nc.gpsimd
                    eng.tensor_add(xr_t[p][e][:], xr_t[p][e][:], pos2r[e][:])
                    nc.gpsimd.tensor_add(xi_t[p][e][:], xi_t[p][e][:], pos2i[e][:])
                    eng.tensor_add(xs_t[p][e][:], xr_t[p][e][:], xi_t[p][e][:])

            # ============ PHASE A: k, logits, v ============
            with tc.tile_pool(name="psA", bufs=1, space="PSUM") as psA:
                # ---- kT [128f, 2b*256s] per (pair, u): Karatsuba ----
                nt = 0
                for p in range(NPAIR):
                    for u in range(4):
                        fs = slice(u * 128, (u + 1) * 128)
                        t1 = psA.tile([128, 512], F32, tag="t1", bufs=2,
                                      name=f"kt1_{p}_{u}")
                        t2 = psA.tile([128, 512], F32, tag="t2", bufs=2,
                                      name=f"kt2_{p}_{u}")
                        t3 = psA.tile([128, 512], F32, tag="t3", bufs=2,
                                      name=f"kt3_{p}_{u}")
                        for e in range(4):
                            nc.tensor.matmul(t1[:], w_r[e][:, fs], xr_t[p][e][:],
                                             start=(e == 0), stop=(e == 3))
                        for e in range(4):
                            nc.tensor.matmul(t2[:], w_i[e][:, fs], xi_t[p][e][:],
                                             start=(e == 0), stop=(e == 3))
                        for e in range(4):
                            nc.tensor.matmul(t3[:], w_s[e][:, fs], xs_t[p][e][:],
                                             start=(e == 0), stop=(e == 3))
                        # vector ops may read at most one PSUM operand, so
                        # t2 goes through SBUF on the scalar engine first
                        sc, c2 = s12[nt % 2], c2s[nt % 2]
                        nc.scalar.copy(c2[:], t2[:])
                        nc.vector.tensor_sub(kTr[p][u][:], t1[:], c2[:])
                        nc.vector.tensor_add(sc[:], t1[:], c2[:])
                        nc.vector.tensor_sub(kTi[p][u][:], t3[:], sc[:])
                        nt += 1

                # ---- logits [8h, 256s] per batch; exp from PSUM ----
                for b in range(BPC):
                    p, hf = divmod(b, 2)
                    cs = slice(hf * 256, (hf + 1) * 256)
                    bs = slice(b * 8, (b + 1) * 8)
                    lr = psA.tile([8, 256], F32, tag="lr", name=f"lr{b}")
                    li = psA.tile([8, 256], F32, tag="li", name=f"li{b}")
                    for u in range(4):
                        nc.tensor.matmul(lr[:], bd_r[u][:, bs], kTr[p][u][:, cs],
                                         start=(u == 0), stop=False)
                        nc.tensor.matmul(lr[:], bd_in[u][:, bs], kTi[p][u][:, cs],
                                         start=False, stop=(u == 3))
                    for u in range(4):
                        nc.tensor.matmul(li[:], bd_r[u][:, bs], kTi[p][u][:, cs],
                                         start=(u == 0), stop=False)
                        nc.tensor.matmul(li[:], bd_i[u][:, bs], kTr[p][u][:, cs],
                                         start=False, stop=(u == 3))
                    nc.scalar.activation(wexp[b][:, 0, :], lr[:], ACTF.Exp,
                                         bias=0.0, scale=1.0,
                                         accum_out=den8[b][:, 0:1])
                    nc.scalar.activation(wexp[b][:, 1, :], li[:], ACTF.Exp,
                                         bias=0.0, scale=1.0,
                                         accum_out=den8[b][:, 1:2])
                    # engines can't write at partition offset b*8; DMA can
                    nc.scalar.dma_start(out=w_sm[bs, :, 1:S], in_=wexp[b][:])
                    nc.scalar.dma_start(out=den[bs, :], in_=den8[b][:])

                # ---- softmax tail: s=0 column + normalization ----
                nc.scalar.activation(e0[:], lg0[:], ACTF.Exp, bias=0.0, scale=1.0)
                nc.vector.tensor_copy(w_sm[:, :, 0], e0[:])
                nc.vector.tensor_add(den2[:], den[:], e0[:])
                nc.vector.reciprocal(rs[:], den2[:])
                nc.vector.tensor_scalar_mul(w_sm[:, 0, :], w_sm[:, 0, :], rs[:, 0:1])
                nc.vector.tensor_scalar_mul(w_sm[:, 1, :], w_sm[:, 1, :], rs[:, 1:2])

                # ---- v [128s, 512f] per (batch, s-block): Karatsuba ----
                for b in range(BPC):
                    p, hf = divmod(b, 2)
                    for st in range(2):
                        scs = slice(hf * 256 + st * 128, hf * 256 + (st + 1) * 128)
                        t1 = psA.tile([128, 512], F32, tag="t1", bufs=2,
                                      name=f"vt1_{b}_{st}")
                        t2 = psA.tile([128, 512], F32, tag="t2", bufs=2,
                                      name=f"vt2_{b}_{st}")
                        t3 = psA.tile([128, 512], F32, tag="t3", bufs=2,
                                      name=f"vt3_{b}_{st}")
                        for e in range(4):
                            nc.tensor.matmul(t1[:], xr_t[p][e][:, scs],
                                             w_r[e][:, VS],
                                             start=(e == 0), stop=(e == 3))
                        for e in range(4):
                            nc.tensor.matmul(t2[:], xi_t[p][e][:, scs],
                                             w_i[e][:, VS],
                                             start=(e == 0), stop=(e == 3))
                        for e in range(4):
                            nc.tensor.matmul(t3[:], xs_t[p][e][:, scs],
                                             w_s[e][:, VS],
                                             start=(e == 0), stop=(e == 3))
                        sc, c2 = s12[nt % 2], c2s[nt % 2]
                        nc.scalar.copy(c2[:], t2[:])
                        nc.vector.tensor_sub(vr[b][st][:], t1[:], c2[:])
                        nc.vector.tensor_add(sc[:], t1[:], c2[:])
                        nc.vector.tensor_sub(vi[b][st][:], t3[:], sc[:])
                        nt += 1

            # ============ PHASE B: wT, hv, extract, y ============
            with tc.tile_pool(name="psT", bufs=1, space="PSUM") as psT:
                # transpose softmax weights -> [128s, 32bh]
                for a in range(2):
                    cs = slice(1 + a * 128, 1 + (a + 1) * 128)
                    ptr = psT.tile([128, 32], BF, tag="tw", bufs=2, name=f"ptr{a}")
                    pti = psT.tile([128, 32], BF, tag="tx", bufs=2, name=f"pti{a}")
                    nc.tensor.transpose(ptr[:], w_sm[:, 0, cs], id32[:])
                    nc.tensor.transpose(pti[:], w_sm[:, 1, cs], id32[:])
                    nc.scalar.copy(wTr[a][:], ptr[:])
                    nc.scalar.copy(wTi[a][:], pti[:])
                    nc.scalar.activation(wTin[a][:], pti[:], ACTF.Copy,
                                         bias=0.0, scale=-1.0)
                # s=0 row of both parts in one [32, 2] -> [2, 32] transpose
                ptc = psT.tile([2, 32], BF, tag="tc")
                nc.tensor.transpose(ptc[:], w_sm[:, :, 0], id32[:])
                nc.scalar.copy(wt0b[:], ptc[:])
                nc.scalar.activation(wt0n[:], ptc[:], ACTF.Copy,
                                     bias=0.0, scale=-1.0)
                nc.sync.dma_start(out=wt0a[0:1, :], in_=wt0b[0:1, :])
                nc.sync.dma_start(out=wt0a[1:2, :], in_=wt0n[1:2, :])

            with tc.tile_pool(name="psH", bufs=1, space="PSUM") as psH:
                # ---- hv: per batch [8h, 512f] ----
                for b in range(BPC):
                    bs = slice(b * 8, (b + 1) * 8)
                    ph_r = psH.tile([NH, OUT], F32, tag="hr", bufs=2, name=f"phr{b}")
                    ph_i = psH.tile([NH, OUT], F32, tag="hi", bufs=2, name=f"phi{b}")
                    mm = nc.tensor.matmul
                    mm(ph_r[:], wTr[0][:, bs], vr[b][0][:], start=True, stop=False)
                    mm(ph_r[:], wTr[1][:, bs], vr[b][1][:], start=False, stop=False)
                    mm(ph_r[:], wTin[0][:, bs], vi[b][0][:], start=False, stop=False)
                    mm(ph_r[:], wTin[1][:, bs], vi[b][1][:], start=False, stop=False)
                    mm(ph_r[:], wt0a[:, bs], v02[:, b, :], start=False, stop=True)
                    mm(ph_i[:], wTi[0][:, bs], vr[b][0][:], start=True, stop=False)
                    mm(ph_i[:], wTi[1][:, bs], vr[b][1][:], start=False, stop=False)
                    mm(ph_i[:], wTr[0][:, bs], vi[b][0][:], start=False, stop=False)
                    mm(ph_i[:], wTr[1][:, bs], vi[b][1][:], start=False, stop=False)
                    mm(ph_i[:], wt0b[:, bs], v02s[:, b, :], start=False, stop=True)
                    nc.vector.tensor_mul(hvm_rb[b][:], ph_r[:], mask8[:])
                    nc.vector.tensor_mul(hvm_ib[b][:], ph_i[:], mask8[:])
                    nc.sync.dma_start(out=hvm_r[bs, :], in_=hvm_rb[b][:])
                    nc.gpsimd.dma_start(out=hvm_i[bs, :], in_=hvm_ib[b][:])

            with tc.tile_pool(name="psY", bufs=1, space="PSUM") as psY:
                # ---- extract attn0^T [128f, 4b] via selection matmul ----
                for u in range(4):
                    fs = slice(u * 128, (u + 1) * 128)
                    par = psY.tile([128, BPC], F32, tag="par", bufs=2, name=f"par{u}")
                    pai = psY.tile([128, BPC], F32, tag="pai", bufs=2, name=f"pai{u}")
                    nc.tensor.matmul(par[:], hvm_r[:, fs], sel32[:],
                                     start=True, stop=True)
                    nc.tensor.matmul(pai[:], hvm_i[:, fs], sel32[:],
                                     start=True, stop=True)
                    nc.scalar.copy(att_r[u][:], par[:])
                    nc.scalar.copy(att_i[u][:], pai[:])
                    nc.scalar.activation(att_in[u][:], pai[:], ACTF.Copy,
                                         bias=0.0, scale=-1.0)

                # ---- y = attn0 @ Wc^T ----
                py_r = psY.tile([BPC, OUT], F32, tag="pyr")
                py_i = psY.tile([BPC, OUT], F32, tag="pyi")
                for j, u in enumerate(range(4)):
                    nc.tensor.matmul(py_r[:], att_r[u][:], wcr[u][:],
                                     start=(j == 0), stop=False)
                    nc.tensor.matmul(py_r[:], att_in[u][:], wci[u][:],
                                     start=False, stop=(j == 3))
                    nc.tensor.matmul(py_i[:], att_r[u][:], wci[u][:],
                                     start=(j == 0), stop=False)
                    nc.tensor.matmul(py_i[:], att_i[u][:], wcr[u][:],
                                     start=False, stop=(j == 3))
                nc.scalar.copy(y_r[:], py_r[:])
                nc.scalar.copy(y_i[:], py_i[:])
                nc.sync.dma_start(out=d_yr.ap(), in_=y_r[:])
                nc.sync.dma_start(out=d_yi.ap(), in_=y_i[:])

    nc.compile()
    return nc


def _host_prep(inputs):
    """Host-side math + per-core in_maps."""
    f32 = np.float32
    xr = np.ascontiguousarray(inputs["x_real"], dtype=f32).reshape(B, E, HW)
    xi = np.ascontiguousarray(inputs["x_imag"], dtype=f32).reshape(B, E, HW)
    pos = np.asarray(inputs["pos_r"], f32) + 1j * np.asarray(inputs["pos_i"], f32)
    w_in = np.asarray(inputs["w_in_r"], f32) + 1j * np.asarray(inputs["w_in_i"], f32)
    b_in = np.asarray(inputs["b_in_r"], f32) + 1j * np.asarray(inputs["b_in_i"], f32)
    w_out = np.asarray(inputs["w_out_r"], f32) + 1j * np.asarray(inputs["w_out_i"], f32)
    b_out = np.asarray(inputs["b_out_r"], f32) + 1j * np.asarray(inputs["b_out_i"], f32)
    w_p = np.asarray(inputs["w_p_r"], f32) + 1j * np.asarray(inputs["w_p_i"], f32)
    b_p = np.asarray(inputs["b_p_r"], f32) + 1j * np.asarray(inputs["b_p_i"], f32)

    # ---- host math for the s=0 (mean) token ----
    x0 = (xr.mean(-1, dtype=np.float64) + 1j * xi.mean(-1, dtype=np.float64)
          ).astype(np.complex64) + pos[:, 0]                     # [B, E]
    qs = 1.0 / np.sqrt(HD)
    q0 = (x0 @ w_in[:E].T + b_in[:E]) * qs                       # [B, E]
    k0 = x0 @ w_in[E:2 * E].T                                    # [B, E]
    v0 = x0 @ w_in[2 * E:].T                                     # [B, E]
    lg0c = np.einsum("bhd,bhd->bh", q0.reshape(B, NH, HD),
                     k0.reshape(B, NH, HD))                      # [B, NH]

    wc = w_p @ w_out                                             # [OUT, E]
    # v-bias exits via sum(softmax)=1; out/proj biases are linear offsets.
    b_v = b_in[2 * E:]
    y_bias = ((1 + 1j) * b_v) @ wc.T + b_out @ w_p.T + b_p       # [OUT]

    bf = lambda a: np.ascontiguousarray(a, dtype=f32).astype(BF16)
    pos2 = np.concatenate([pos[:, 1:S], pos[:, 1:S]], axis=1)    # [E, 512]
    wkv = w_in[E:].T                                             # [E, 2E] complex
    shared = dict(
        posr2=bf(pos2.real), posi2=bf(pos2.imag),
        wr=bf(wkv.real), wi=bf(wkv.imag), ws=bf(wkv.real + wkv.imag),
        wcr=bf(wc.real.T), wci=bf(wc.imag.T),
        id32=np.eye(32, dtype=f32).astype(BF16),
    )
    mask8 = np.zeros((NH, OUT), f32)
    for h in range(NH):
        mask8[h, h * HD:(h + 1) * HD] = 1.0
    sel32 = np.zeros((32, BPC), f32)
    for b in range(BPC):
        sel32[b * 8:(b + 1) * 8, b] = 1.0
    shared["mask8"] = mask8.astype(BF16)
    shared["sel32"] = sel32.astype(BF16)

    in_maps = []
    for c in range(NCORES):
        bsl = slice(c * BPC, (c + 1) * BPC)
        q0c, v0c, lg0c_c = q0[bsl], v0[bsl], lg0c[bsl]
        # block-diag bd [E, 32]: row f (grouped per u-tile), col b*8 + h(f)
        bdr = np.zeros((E, 32), f32)
        bdi = np.zeros((E, 32), f32)
        fidx = np.arange(E)
        for b in range(BPC):
            bdr[fidx, b * 8 + fidx // HD] = q0c[b].real
            bdi[fidx, b * 8 + fidx // HD] = q0c[b].imag
        lg0m = np.empty((32, 2), f32)
        lg0m[:, 0] = lg0c_c.real.reshape(-1)
        lg0m[:, 1] = lg0c_c.imag.reshape(-1)
        v02 = np.stack([v0c.real.astype(f32), v0c.imag.astype(f32)])  # [2,BPC,OUT]
        m = dict(shared)
        m["bdr"] = bdr.astype(BF16)
        m["bdi"] = bdi.astype(BF16)
        m["bdin"] = (-bdi).astype(BF16)
        m["lg0"] = lg0m
        m["v02"] = v02.astype(BF16)
        m["v02s"] = v02[::-1].copy().astype(BF16)
        m["xr"] = xr[bsl].astype(BF16)
        m["xi"] = xi[bsl].astype(BF16)
        in_maps.append(m)
    return in_maps, y_bias.astype(np.complex64)


def _run(inputs, trace=False, **kw):
    from concourse.bass_utils import run_bass_kernel_spmd
    if "nc" not in _cached:
        _cached["nc"] = _build()
    nc = _cached["nc"]
    in_maps, y_bias = _host_prep(inputs)
    res = run_bass_kernel_spmd(nc, in_maps, core_ids=list(range(NCORES)),
                               trace=trace, **kw)
    out = np.empty((B, OUT), np.complex64)
    for c in range(NCORES):
        out[c * BPC:(c + 1) * BPC] = (res.results[c]["yr"]
                                      + 1j * res.results[c]["yi"])
    out += y_bias[None, :]
    return out, res


def kernel(**inputs) -> np.ndarray:
    out, _ = _run(inputs)
    return out


# revision 16
# speedup vs baseline: 1.3709x; 1.0414x over previous
"""Complex AttentionPool2d on 8 trn2 NeuronCores, data-parallel over batch.

Contract: kernel(**inputs) takes the FULL inputs from setup_inputs() and
returns the FULL [32, 512] complex64 output.

v3 design: all matmuls in bf16 (tolerance is 2e-2), Karatsuba (3 real
matmuls) for the dominant k/v projections, and every small/serial piece of
math moved to the host:
  host: pos folded into the shipped x (x' = pixels + pos, pre-paired
        [pair, E, 2b, 256s]); x0 = mean(x)+pos0; q0 = (x0 Wq + b_q)/8;
        k0 = x0 Wk; v0 = x0 Wv; logit[s=0] = q0.k0; block-diag bd tiles
        from q0; v0 row-pairs; y-bias added to the final output on host.
  device (per core, 4 batches as 2 column-packed pairs):
        kT[f, (b,s)]   = Wk @ x'                 # Karatsuba, f-major
        v[(b,st)]      = x'^T @ Wv               # Karatsuba, s-major
        logits[8h,256] = bd^T @ kT per batch; col 0 from host
        w = softmax(re) + i softmax(im)          # exp straight from PSUM
        attn0 = w^T v (+ w0 x v0 row term)       # per batch [8, 512]
        y = attn0 @ (w_p w_out)^T                # via sel-extracted att^T

Engine rules honored: GPSIMD can't touch PSUM; vector ops read at most one
PSUM operand (stage via scalar-engine copies); compute engines can't write
at non-32-aligned partition offsets (assemble via DMA).

Math identities: k-bias dropped (softmax invariant); v-bias exits through
sum(w)=1 as a constant y-offset (host-added); q-bias folded into host q0.
"""
import contextlib
import numpy as np
import ml_dtypes

B, E, HW, S = 32, 512, 256, 257
NH, HD = 8, 64
OUT = 512
NCORES = 8
BPC = B // NCORES   # batches per core
NPAIR = BPC // 2    # column-packed batch pairs
BF16 = ml_dtypes.bfloat16

_cached = {}


def _build():
    import concourse.bacc as bacc
    import concourse.tile as tile
    import concourse.mybir as mybir

    F32 = mybir.dt.float32
    BF = mybir.dt.bfloat16
    ACTF = mybir.ActivationFunctionType

    nc = bacc.Bacc("TRN2", target_bir_lowering=False, debug=False)

    # ---- DRAM I/O ----
    d_xr = nc.dram_tensor("xr", [NPAIR, E, 2, HW], BF, kind="ExternalInput")
    d_xi = nc.dram_tensor("xi", [NPAIR, E, 2, HW], BF, kind="ExternalInput")
    d_wr = nc.dram_tensor("wr", [E, 2 * E], BF, kind="ExternalInput")
    d_wi = nc.dram_tensor("wi", [E, 2 * E], BF, kind="ExternalInput")
    d_ws = nc.dram_tensor("ws", [E, 2 * E], BF, kind="ExternalInput")
    d_wcr = nc.dram_tensor("wcr", [E, OUT], BF, kind="ExternalInput")
    d_wci = nc.dram_tensor("wci", [E, OUT], BF, kind="ExternalInput")
    d_bdr = nc.dram_tensor("bdr", [E, 32], BF, kind="ExternalInput")
    d_bdi = nc.dram_tensor("bdi", [E, 32], BF, kind="ExternalInput")
    d_bdin = nc.dram_tensor("bdin", [E, 32], BF, kind="ExternalInput")
    d_lg0 = nc.dram_tensor("lg0", [32, 2], F32, kind="ExternalInput")
    d_v02 = nc.dram_tensor("v02", [2, BPC, OUT], BF, kind="ExternalInput")
    d_v02s = nc.dram_tensor("v02s", [2, BPC, OUT], BF, kind="ExternalInput")
    d_id32 = nc.dram_tensor("id32", [32, 32], BF, kind="ExternalInput")
    d_mask = nc.dram_tensor("mask8", [NH, OUT], BF, kind="ExternalInput")
    d_sel = nc.dram_tensor("sel32", [32, BPC], BF, kind="ExternalInput")
    d_yr = nc.dram_tensor("yr", [BPC, OUT], F32, kind="ExternalOutput")
    d_yi = nc.dram_tensor("yi", [BPC, OUT], F32, kind="ExternalOutput")

    KS = slice(0, 512)       # k columns of the packed kv weight
    VS = slice(512, 1024)    # v columns

    with tile.TileContext(nc) as tc, \
         nc.allow_low_precision(reason="bf16 kernel; tolerance is 2e-2"):
        with tc.tile_pool(name="consts", bufs=1) as consts, \
             tc.tile_pool(name="keep", bufs=1) as keep:
            # ---- persistent weights / constants ----
            w_r = [consts.tile([128, 2 * E], BF, name=f"wr{e}") for e in range(4)]
            w_i = [consts.tile([128, 2 * E], BF, name=f"wi{e}") for e in range(4)]
            w_s = [consts.tile([128, 2 * E], BF, name=f"ws{e}") for e in range(4)]
            wcr = [consts.tile([128, OUT], BF, name=f"wcr{e}") for e in range(4)]
            wci = [consts.tile([128, OUT], BF, name=f"wci{e}") for e in range(4)]
            bd_r = [consts.tile([128, 32], BF, name=f"bdr{u}") for u in range(4)]
            bd_i = [consts.tile([128, 32], BF, name=f"bdi{u}") for u in range(4)]
            bd_in = [consts.tile([128, 32], BF, name=f"bdin{u}") for u in range(4)]
            lg0 = consts.tile([32, 2], F32)
            v02 = consts.tile([2, BPC, OUT], BF)
            v02s = consts.tile([2, BPC, OUT], BF)
            id32 = consts.tile([32, 32], BF)
            mask8 = consts.tile([NH, OUT], BF)
            sel32 = consts.tile([32, BPC], BF)

            # ---- persistent activations ----
            xr_t = [[keep.tile([128, 512], BF, name=f"xr{p}_{e}")
                     for e in range(4)] for p in range(NPAIR)]
            xi_t = [[keep.tile([128, 512], BF, name=f"xi{p}_{e}")
                     for e in range(4)] for p in range(NPAIR)]
            xs_t = [[keep.tile([128, 512], BF, name=f"xs{p}_{e}")
                     for e in range(4)] for p in range(NPAIR)]
            kTr = [[keep.tile([128, 512], BF, name=f"kTr{p}_{u}")
                    for u in range(4)] for p in range(NPAIR)]
            kTi = [[keep.tile([128, 512], BF, name=f"kTi{p}_{u}")
                    for u in range(4)] for p in range(NPAIR)]
            vr = [[keep.tile([128, OUT], BF, name=f"vr{b}_{s}")
                   for s in range(2)] for b in range(BPC)]
            vi = [[keep.tile([128, OUT], BF, name=f"vi{b}_{s}")
                   for s in range(2)] for b in range(BPC)]
            w_sm = keep.tile([32, 2, S], BF)        # softmax weights (re|im)
            wexp = [keep.tile([NH, 2, 256], BF, name=f"wexp{b}")
                    for b in range(BPC)]            # per-batch exp staging
            den8 = [keep.tile([NH, 2], F32, name=f"den8_{b}") for b in range(BPC)]
            den = keep.tile([32, 2], F32)           # exp row-sums of cols 1..256
            e0 = keep.tile([32, 2], F32)            # exp of the s=0 logit
            den2 = keep.tile([32, 2], F32)
            rs = keep.tile([32, 2], F32)
            wTr = [keep.tile([128, 32], BF, name=f"wTr{a}") for a in range(2)]
            wTi = [keep.tile([128, 32], BF, name=f"wTi{a}") for a in range(2)]
            wTin = [keep.tile([128, 32], BF, name=f"wTin{a}") for a in range(2)]
            wt0a = keep.tile([2, 32], BF)           # rows (w0r, -w0i)
            wt0b = keep.tile([2, 32], BF)           # rows (w0r, w0i)
            wt0n = keep.tile([2, 32], BF)
            hvm_r = keep.tile([32, OUT], BF)
            hvm_i = keep.tile([32, OUT], BF)
            hvm_rb = [keep.tile([NH, OUT], BF, name=f"hvr{b}") for b in range(BPC)]
            hvm_ib = [keep.tile([NH, OUT], BF, name=f"hvi{b}") for b in range(BPC)]
            att_r = [keep.tile([128, BPC], BF, name=f"atr{u}") for u in range(4)]
            att_i = [keep.tile([128, BPC], BF, name=f"ati{u}") for u in range(4)]
            att_in = [keep.tile([128, BPC], BF, name=f"atn{u}") for u in range(4)]
            s12 = [keep.tile([128, 512], F32, name=f"s12_{j}") for j in range(2)]
            c2s = [keep.tile([128, 512], F32, name=f"c2_{j}") for j in range(2)]
            c1s = [keep.tile([128, 512], F32, name=f"c1_{j}") for j in range(2)]
            y_r = keep.tile([BPC, OUT], F32)
            y_i = keep.tile([BPC, OUT], F32)

            # ---- DMA emission; first-needed bytes first ----
            # k-weights split across sync+scalar queues so they land fastest
            for d_w, w_t in ((d_wr, w_r), (d_wi, w_i), (d_ws, w_s)):
                for e in range(4):
                    sl = slice(e * 128, (e + 1) * 128)
                    q = nc.sync if e < 2 else nc.scalar
                    q.dma_start(out=w_t[e][:, KS], in_=d_w.ap()[sl, KS])
            # x: one DMA per (pair, e, part) thanks to host pre-pairing
            for p in range(NPAIR):
                for e in range(4):
                    sl = slice(e * 128, (e + 1) * 128)
                    nc.gpsimd.dma_start(out=xr_t[p][e][:],
                                        in_=d_xr.ap()[p, sl, :, :])
                    nc.gpsimd.dma_start(out=xi_t[p][e][:],
                                        in_=d_xi.ap()[p, sl, :, :])
            # smalls on scalar queue
            for u in range(4):
                sl = slice(u * 128, (u + 1) * 128)
                nc.scalar.dma_start(out=bd_r[u], in_=d_bdr.ap()[sl, :])
                nc.scalar.dma_start(out=bd_in[u], in_=d_bdin.ap()[sl, :])
                nc.scalar.dma_start(out=bd_i[u], in_=d_bdi.ap()[sl, :])
            nc.scalar.dma_start(out=lg0, in_=d_lg0.ap())
            nc.scalar.dma_start(out=v02, in_=d_v02.ap())
            nc.scalar.dma_start(out=v02s, in_=d_v02s.ap())
            nc.scalar.dma_start(out=id32, in_=d_id32.ap())
            nc.scalar.dma_start(out=mask8, in_=d_mask.ap())
            nc.scalar.dma_start(out=sel32, in_=d_sel.ap())
            # v-weights + wc stream in under the k phase
            for d_w, w_t in ((d_wr, w_r), (d_wi, w_i), (d_ws, w_s)):
                for e in range(4):
                    sl = slice(e * 128, (e + 1) * 128)
                    nc.sync.dma_start(out=w_t[e][:, VS], in_=d_w.ap()[sl, VS])
            for e in range(4):
                sl = slice(e * 128, (e + 1) * 128)
                nc.sync.dma_start(out=wcr[e], in_=d_wcr.ap()[sl, :])
                nc.sync.dma_start(out=wci[e], in_=d_wci.ap()[sl, :])

            # Karatsuba sums of x on vector (SBUF-only, early)
            for p in range(NPAIR):
                for e in range(4):
                    nc.vector.tensor_add(xs_t[p][e][:], xr_t[p][e][:],
                                         xi_t[p][e][:])

            stL = contextlib.ExitStack()
            psL = stL.enter_context(
                tc.tile_pool(name="psL", bufs=1, space="PSUM"))
            st = contextlib.ExitStack()
            psK = st.enter_context(
                tc.tile_pool(name="psK", bufs=1, space="PSUM"))

            nt = 0

            def k_uhalf(p, uh):
                nonlocal nt
                us = (2 * uh, 2 * uh + 1)
                tl = {}
                for kind, w_k, x_k in (("t1", w_r, xr_t), ("t2", w_i, xi_t),
                                       ("t3", w_s, xs_t)):
                    for u in us:
                        fs = slice(u * 128, (u + 1) * 128)
                        t = psK.tile([128, 512], F32, tag=kind, bufs=2,
                                     name=f"k{kind}_{p}_{u}")
                        tl[(kind, u)] = t
                        for e in range(4):
                            nc.tensor.matmul(t[:], w_k[e][:, fs], x_k[p][e][:],
                                             start=(e == 0), stop=(e == 3))
                for u in us:
                    t1, t2, t3 = tl[("t1", u)], tl[("t2", u)], tl[("t3", u)]
                    sc, c2 = s12[nt % 2], c2s[nt % 2]
                    nc.scalar.copy(c2[:], t2[:])
                    nc.vector.tensor_sub(kTr[p][u][:], t1[:], c2[:])
                    nc.vector.tensor_add(sc[:], t1[:], c2[:])
                    nc.vector.tensor_sub(kTi[p][u][:], t3[:], sc[:])
                    nt += 1

            def logits_batch(b):
                p, hf = divmod(b, 2)
                cs = slice(hf * 256, (hf + 1) * 256)
                bs = slice(b * 8, (b + 1) * 8)
                lr = psL.tile([8, 256], F32, tag="lr", name=f"lr{b}")
                li = psL.tile([8, 256], F32, tag="li", name=f"li{b}")
                for u in range(4):
                    nc.tensor.matmul(lr[:], bd_r[u][:, bs], kTr[p][u][:, cs],
                                     start=(u == 0), stop=False)
                    nc.tensor.matmul(lr[:], bd_in[u][:, bs], kTi[p][u][:, cs],
                                     start=False, stop=(u == 3))
                for u in range(4):
                    nc.tensor.matmul(li[:], bd_r[u][:, bs], kTi[p][u][:, cs],
                                     start=(u == 0), stop=False)
                    nc.tensor.matmul(li[:], bd_i[u][:, bs], kTr[p][u][:, cs],
                                     start=False, stop=(u == 3))
                nc.scalar.activation(wexp[b][:, 0, :], lr[:], ACTF.Exp,
                                     bias=0.0, scale=1.0,
                                     accum_out=den8[b][:, 0:1])
                nc.scalar.activation(wexp[b][:, 1, :], li[:], ACTF.Exp,
                                     bias=0.0, scale=1.0,
                                     accum_out=den8[b][:, 1:2])
                # engines can't write at partition offset b*8; DMA can
                nc.scalar.dma_start(out=w_sm[bs, :, 1:S], in_=wexp[b][:])
                nc.scalar.dma_start(out=den[bs, :], in_=den8[b][:])

            # ---- k + logits, interleaved so PE never waits on combines ----
            k_uhalf(0, 0)
            k_uhalf(0, 1)
            k_uhalf(1, 0)
            logits_batch(0)
            logits_batch(1)
            k_uhalf(1, 1)
            logits_batch(2)
            logits_batch(3)
            st.close()   # psK

            # ---- softmax tail: s=0 column + normalization ----
            nc.scalar.activation(e0[:], lg0[:], ACTF.Exp, bias=0.0, scale=1.0)
            nc.vector.tensor_copy(w_sm[:, :, 0], e0[:])
            nc.vector.tensor_add(den2[:], den[:], e0[:])
            nc.vector.reciprocal(rs[:], den2[:])
            nc.vector.tensor_scalar_mul(w_sm[:, 0, :], w_sm[:, 0, :], rs[:, 0:1])
            nc.vector.tensor_scalar_mul(w_sm[:, 1, :], w_sm[:, 1, :], rs[:, 1:2])
            stL.close()  # psL

            stV = contextlib.ExitStack()
            psV = stV.enter_context(
                tc.tile_pool(name="psV", bufs=1, space="PSUM"))

            def v_batch(b):
                nonlocal nt
                p, hf = divmod(b, 2)
                for stt in range(2):
                    scs = slice(hf * 256 + stt * 128,
                                hf * 256 + (stt + 1) * 128)
                    tl = {}
                    for kind, bufs, w_off, x_k in (
                            ("t1", 2, w_r, xr_t), ("t2", 1, w_i, xi_t),
                            ("t3", 1, w_s, xs_t)):
                        t = psV.tile([128, 512], F32, tag=kind, bufs=bufs,
                                     name=f"v{kind}_{b}_{stt}")
                        tl[kind] = t
                        for e in range(4):
                            nc.tensor.matmul(t[:], x_k[p][e][:, scs],
                                             w_off[e][:, VS],
                                             start=(e == 0), stop=(e == 3))
                    # offload the SBUF-only part of this combine to gpsimd
                    c1, c2, sc = c1s[nt % 2], c2s[nt % 2], s12[nt % 2]
                    nc.scalar.copy(c1[:], tl["t1"][:])
                    nc.scalar.copy(c2[:], tl["t2"][:])
                    nc.gpsimd.tensor_sub(vr[b][stt][:], c1[:], c2[:])
                    nc.gpsimd.tensor_add(sc[:], c1[:], c2[:])
                    nc.vector.tensor_sub(vi[b][stt][:], tl["t3"][:], sc[:])
                    nt += 1

            def hv_batch(b, psH):
                bs = slice(b * 8, (b + 1) * 8)
                ph_r = psH.tile([NH, OUT], F32, tag="hr", bufs=2, name=f"phr{b}")
                ph_i = psH.tile([NH, OUT], F32, tag="hi", bufs=2, name=f"phi{b}")
                mm = nc.tensor.matmul
                mm(ph_r[:], wTr[0][:, bs], vr[b][0][:], start=True, stop=False)
                mm(ph_r[:], wTr[1][:, bs], vr[b][1][:], start=False, stop=False)
                mm(ph_r[:], wTin[0][:, bs], vi[b][0][:], start=False, stop=False)
                mm(ph_r[:], wTin[1][:, bs], vi[b][1][:], start=False, stop=False)
                mm(ph_r[:], wt0a[:, bs], v02[:, b, :], start=False, stop=True)
                mm(ph_i[:], wTi[0][:, bs], vr[b][0][:], start=True, stop=False)
                mm(ph_i[:], wTi[1][:, bs], vr[b][1][:], start=False, stop=False)
                mm(ph_i[:], wTr[0][:, bs], vi[b][0][:], start=False, stop=False)
                mm(ph_i[:], wTr[1][:, bs], vi[b][1][:], start=False, stop=False)
                mm(ph_i[:], wt0b[:, bs], v02s[:, b, :], start=False, stop=True)
                nc.vector.tensor_mul(hvm_rb[b][:], ph_r[:], mask8[:])
                nc.vector.tensor_mul(hvm_ib[b][:], ph_i[:], mask8[:])
                nc.sync.dma_start(out=hvm_r[bs, :], in_=hvm_rb[b][:])
                nc.sync.dma_start(out=hvm_i[bs, :], in_=hvm_ib[b][:])

            v_batch(0)

            # ---- softmax-weight transposes -> [128s, 32bh] ----
            with tc.tile_pool(name="psT", bufs=1, space="PSUM") as psT:
                for a in range(2):
                    cs = slice(1 + a * 128, 1 + (a + 1) * 128)
                    ptr = psT.tile([128, 32], BF, tag="tw", name=f"ptr{a}")
                    pti = psT.tile([128, 32], BF, tag="tx", name=f"pti{a}")
                    nc.tensor.transpose(ptr[:], w_sm[:, 0, cs], id32[:])
                    nc.tensor.transpose(pti[:], w_sm[:, 1, cs], id32[:])
                    nc.scalar.copy(wTr[a][:], ptr[:])
                    nc.scalar.copy(wTi[a][:], pti[:])
                    nc.scalar.activation(wTin[a][:], pti[:], ACTF.Copy,
                                         bias=0.0, scale=-1.0)
                # s=0 row of both parts in one [32, 2] -> [2, 32] transpose
                ptc = psT.tile([2, 32], BF, tag="tc")
                nc.tensor.transpose(ptc[:], w_sm[:, :, 0], id32[:])
                nc.scalar.copy(wt0b[:], ptc[:])
                nc.scalar.activation(wt0n[:], ptc[:], ACTF.Copy,
                                     bias=0.0, scale=-1.0)
                nc.sync.dma_start(out=wt0a[0:1, :], in_=wt0b[0:1, :])
                nc.sync.dma_start(out=wt0a[1:2, :], in_=wt0n[1:2, :])

            stH = contextlib.ExitStack()
            psH = stH.enter_context(
                tc.tile_pool(name="psH", bufs=1, space="PSUM"))
            v_batch(1)
            hv_batch(0, psH)
            v_batch(2)
            hv_batch(1, psH)
            v_batch(3)
            hv_batch(2, psH)
            hv_batch(3, psH)
            stH.close()  # psH (before psY opens; psV closes last)

            with tc.tile_pool(name="psY", bufs=1, space="PSUM") as psY:
                # ---- extract attn0^T [128f, 4b] via selection matmul ----
                for u in range(4):
                    fs = slice(u * 128, (u + 1) * 128)
                    par = psY.tile([128, BPC], F32, tag="par", name=f"par{u}")
                    pai = psY.tile([128, BPC], F32, tag="pai", name=f"pai{u}")
                    nc.tensor.matmul(par[:], hvm_r[:, fs], sel32[:],
                                     start=True, stop=True)
                    nc.tensor.matmul(pai[:], hvm_i[:, fs], sel32[:],
                                     start=True, stop=True)
                    nc.scalar.copy(att_r[u][:], par[:])
                    nc.scalar.copy(att_i[u][:], pai[:])
                    nc.scalar.activation(att_in[u][:], pai[:], ACTF.Copy,
                                         bias=0.0, scale=-1.0)

                # ---- y = attn0 @ Wc^T ----
                py_r = psY.tile([BPC, OUT], F32, tag="pyr")
                py_i = psY.tile([BPC, OUT], F32, tag="pyi")
                for j, u in enumerate(range(4)):
                    nc.tensor.matmul(py_r[:], att_r[u][:], wcr[u][:],
                                     start=(j == 0), stop=False)
                    nc.tensor.matmul(py_r[:], att_in[u][:], wci[u][:],
                                     start=False, stop=(j == 3))
                    nc.tensor.matmul(py_i[:], att_r[u][:], wci[u][:],
                                     start=(j == 0), stop=False)
                    nc.tensor.matmul(py_i[:], att_i[u][:], wcr[u][:],
                                     start=False, stop=(j == 3))
                nc.scalar.copy(y_r[:], py_r[:])
                nc.scalar.copy(y_i[:], py_i[:])
                nc.sync.dma_start(out=d_yr.ap(), in_=y_r[:])
                nc.sync.dma_start(out=d_yi.ap(), in_=y_i[:])

            stV.close()

    nc.compile()
    return nc


def _host_prep(inputs):
    """Host-side math + per-core in_maps."""
    f32 = np.float32
    xr = np.ascontiguousarray(inputs["x_real"], dtype=f32).reshape(B, E, HW)
    xi = np.ascontiguousarray(inputs["x_imag"], dtype=f32).reshape(B, E, HW)
    pos = np.asarray(inputs["pos_r"], f32) + 1j * np.asarray(inputs["pos_i"], f32)
    w_in = np.asarray(inputs["w_in_r"], f32) + 1j * np.asarray(inputs["w_in_i"], f32)
    b_in = np.asarray(inputs["b_in_r"], f32) + 1j * np.asarray(inputs["b_in_i"], f32)
    w_out = np.asarray(inputs["w_out_r"], f32) + 1j * np.asarray(inputs["w_out_i"], f32)
    b_out = np.asarray(inputs["b_out_r"], f32) + 1j * np.asarray(inputs["b_out_i"], f32)
    w_p = np.asarray(inputs["w_p_r"], f32) + 1j * np.asarray(inputs["w_p_i"], f32)
    b_p = np.asarray(inputs["b_p_r"], f32) + 1j * np.asarray(inputs["b_p_i"], f32)

    # ---- host math for the s=0 (mean) token ----
    x0 = (xr.mean(-1, dtype=np.float64) + 1j * xi.mean(-1, dtype=np.float64)
          ).astype(np.complex64) + pos[:, 0]                     # [B, E]
    qs = 1.0 / np.sqrt(HD)
    q0 = (x0 @ w_in[:E].T + b_in[:E]) * qs                       # [B, E]
    k0 = x0 @ w_in[E:2 * E].T                                    # [B, E]
    v0 = x0 @ w_in[2 * E:].T                                     # [B, E]
    lg0c = np.einsum("bhd,bhd->bh", q0.reshape(B, NH, HD),
                     k0.reshape(B, NH, HD))                      # [B, NH]

    wc = w_p @ w_out                                             # [OUT, E]
    # v-bias exits via sum(softmax)=1; out/proj biases are linear offsets.
    b_v = b_in[2 * E:]
    y_bias = ((1 + 1j) * b_v) @ wc.T + b_out @ w_p.T + b_p       # [OUT]

    # pos folded into the shipped x; pre-paired [pair, E, 2b, 256s]
    xr_s = (xr + pos.real[None, :, 1:S]).reshape(NCORES, NPAIR, 2, E, HW)
    xi_s = (xi + pos.imag[None, :, 1:S]).reshape(NCORES, NPAIR, 2, E, HW)
    xr_s = np.ascontiguousarray(xr_s.transpose(0, 1, 3, 2, 4)).astype(BF16)
    xi_s = np.ascontiguousarray(xi_s.transpose(0, 1, 3, 2, 4)).astype(BF16)

    bf = lambda a: np.ascontiguousarray(a, dtype=f32).astype(BF16)
    wkv = w_in[E:].T                                             # [E, 2E] complex
    shared = dict(
        wr=bf(wkv.real), wi=bf(wkv.imag), ws=bf(wkv.real + wkv.imag),
        wcr=bf(wc.real.T), wci=bf(wc.imag.T),
        id32=np.eye(32, dtype=f32).astype(BF16),
    )
    mask8 = np.zeros((NH, OUT), f32)
    for h in range(NH):
        mask8[h, h * HD:(h + 1) * HD] = 1.0
    sel32 = np.zeros((32, BPC), f32)
    for b in range(BPC):
        sel32[b * 8:(b + 1) * 8, b] = 1.0
    shared["mask8"] = mask8.astype(BF16)
    shared["sel32"] = sel32.astype(BF16)

    in_maps = []
    for c in range(NCORES):
        bsl = slice(c * BPC, (c + 1) * BPC)
        q0c, v0c, lg0c_c = q0[bsl], v0[bsl], lg0c[bsl]
        # block-diag bd [E, 32]: row f (grouped per u-tile), col b*8 + h(f)
        bdr = np.zeros((E, 32), f32)
        bdi = np.zeros((E, 32), f32)
        fidx = np.arange(E)
        for b in range(BPC):
            bdr[fidx, b * 8 + fidx // HD] = q0c[b].real
            bdi[fidx, b * 8 + fidx // HD] = q0c[b].imag
        lg0m = np.empty((32, 2), f32)
        lg0m[:, 0] = lg0c_c.real.reshape(-1)
        lg0m[:, 1] = lg0c_c.imag.reshape(-1)
        v02 = np.stack([v0c.real.astype(f32), v0c.imag.astype(f32)])  # [2,BPC,OUT]
        m = dict(shared)
        m["bdr"] = bdr.astype(BF16)
        m["bdi"] = bdi.astype(BF16)
        m["bdin"] = (-bdi).astype(BF16)
        m["lg0"] = lg0m
        m["v02"] = v02.astype(BF16)
        m["v02s"] = v02[::-1].copy().astype(BF16)
        m["xr"] = xr_s[c]
        m["xi"] = xi_s[c]
        in_maps.append(m)
    return in_maps, y_bias.astype(np.complex64)


def _run(inputs, trace=False, **kw):
    from concourse.bass_utils import run_bass_kernel_spmd
    if "nc" not in _cached:
        _cached["nc"] = _build()
    nc = _cached["nc"]
    in_maps, y_bias = _host_prep(inputs)
    res = run_bass_kernel_spmd(nc, in_maps, core_ids=list(range(NCORES)),
                               trace=trace, **kw)
    out = np.empty((B, OUT), np.complex64)
    for c in range(NCORES):
        out[c * BPC:(c + 1) * BPC] = (res.results[c]["yr"]
                                      + 1j * res.results[c]["yi"])
    out += y_bias[None, :]
    return out, res


def kernel(**inputs) -> np.ndarray:
    out, _ = _run(inputs)
    return out


# revision 18
# speedup vs baseline: 1.5508x; 1.1313x over previous
"""Complex AttentionPool2d on 8 trn2 NeuronCores, data-parallel over batch.

Contract: kernel(**inputs) takes the FULL inputs from setup_inputs() and
returns the FULL [32, 512] complex64 output.

v3 design: all matmuls in bf16 (tolerance is 2e-2), Karatsuba (3 real
matmuls) for the dominant k/v projections, and every small/serial piece of
math moved to the host:
  host: pos folded into the shipped x (x' = pixels + pos, pre-paired
        [pair, E, 2b, 256s]); x0 = mean(x)+pos0; q0 = (x0 Wq + b_q)/8;
        k0 = x0 Wk; v0 = x0 Wv; logit[s=0] = q0.k0; block-diag bd tiles
        from q0; v0 row-pairs; y-bias added to the final output on host.
  device (per core, 4 batches as 2 column-packed pairs):
        kT[f, (b,s)]   = Wk @ x'                 # Karatsuba, f-major
        v[(b,st)]      = x'^T @ Wv               # Karatsuba, s-major
        logits[8h,256] = bd^T @ kT per batch; col 0 from host
        w = softmax(re) + i softmax(im)          # exp straight from PSUM
        attn0 = w^T v (+ w0 x v0 row term)       # per batch [8, 512]
        y = attn0 @ (w_p w_out)^T                # via sel-extracted att^T

Engine rules honored: GPSIMD can't touch PSUM; vector ops read at most one
PSUM operand (stage via scalar-engine copies); compute engines can't write
at non-32-aligned partition offsets (assemble via DMA).

Math identities: k-bias dropped (softmax invariant); v-bias exits through
sum(w)=1 as a constant y-offset (host-added); q-bias folded into host q0.
"""
import contextlib
import numpy as np
import ml_dtypes

B, E, HW, S = 32, 512, 256, 257
NH, HD = 8, 64
OUT = 512
NCORES = 8
BPC = B // NCORES   # batches per core
NPAIR = BPC // 2    # column-packed batch pairs
BF16 = ml_dtypes.bfloat16

_cached = {}


def _build():
    import concourse.bacc as bacc
    import concourse.tile as tile
    import concourse.mybir as mybir

    F32 = mybir.dt.float32
    BF = mybir.dt.bfloat16
    ACTF = mybir.ActivationFunctionType

    nc = bacc.Bacc("TRN2", target_bir_lowering=False, debug=False)

    # ---- DRAM I/O ----
    d_xr = nc.dram_tensor("xr", [NPAIR, E, 2, HW], BF, kind="ExternalInput")
    d_xi = nc.dram_tensor("xi", [NPAIR, E, 2, HW], BF, kind="ExternalInput")
    d_wr = nc.dram_tensor("wr", [E, 2 * E], BF, kind="ExternalInput")
    d_wi = nc.dram_tensor("wi", [E, 2 * E], BF, kind="ExternalInput")
    d_ws = nc.dram_tensor("ws", [E, 2 * E], BF, kind="ExternalInput")
    d_wcr = nc.dram_tensor("wcr", [E, OUT], BF, kind="ExternalInput")
    d_wci = nc.dram_tensor("wci", [E, OUT], BF, kind="ExternalInput")
    d_bdr = nc.dram_tensor("bdr", [E, 32], BF, kind="ExternalInput")
    d_bdi = nc.dram_tensor("bdi", [E, 32], BF, kind="ExternalInput")
    d_bdin = nc.dram_tensor("bdin", [E, 32], BF, kind="ExternalInput")
    d_lg0 = nc.dram_tensor("lg0", [32, 2], F32, kind="ExternalInput")
    d_v02 = nc.dram_tensor("v02", [2, BPC, OUT], BF, kind="ExternalInput")
    d_v02s = nc.dram_tensor("v02s", [2, BPC, OUT], BF, kind="ExternalInput")
    d_id32 = nc.dram_tensor("id32", [32, 32], BF, kind="ExternalInput")
    d_mask = nc.dram_tensor("mask8", [NH, OUT], BF, kind="ExternalInput")
    d_sel = nc.dram_tensor("sel32", [32, BPC], BF, kind="ExternalInput")
    d_yr = nc.dram_tensor("yr", [BPC, OUT], F32, kind="ExternalOutput")
    d_yi = nc.dram_tensor("yi", [BPC, OUT], F32, kind="ExternalOutput")

    KS = slice(0, 512)       # k columns of the packed kv weight
    VS = slice(512, 1024)    # v columns

    with tile.TileContext(nc) as tc, \
         nc.allow_low_precision(reason="bf16 kernel; tolerance is 2e-2"):
        with tc.tile_pool(name="consts", bufs=1) as consts, \
             tc.tile_pool(name="keep", bufs=1) as keep:
            # ---- persistent weights / constants ----
            w_r = [consts.tile([128, 2 * E], BF, name=f"wr{e}") for e in range(4)]
            w_i = [consts.tile([128, 2 * E], BF, name=f"wi{e}") for e in range(4)]
            w_s = [consts.tile([128, 2 * E], BF, name=f"ws{e}") for e in range(4)]
            wcr = [consts.tile([128, OUT], BF, name=f"wcr{e}") for e in range(4)]
            wci = [consts.tile([128, OUT], BF, name=f"wci{e}") for e in range(4)]
            bd_r = [consts.tile([128, 32], BF, name=f"bdr{u}") for u in range(4)]
            bd_i = [consts.tile([128, 32], BF, name=f"bdi{u}") for u in range(4)]
            bd_in = [consts.tile([128, 32], BF, name=f"bdin{u}") for u in range(4)]
            lg0 = consts.tile([32, 2], F32)
            v02 = consts.tile([2, BPC, OUT], BF)
            v02s = consts.tile([2, BPC, OUT], BF)
            id32 = consts.tile([32, 32], BF)
            mask8 = consts.tile([NH, OUT], BF)
            sel32 = consts.tile([32, BPC], BF)

            # ---- persistent activations ----
            xr_t = [[keep.tile([128, 512], BF, name=f"xr{p}_{e}")
                     for e in range(4)] for p in range(NPAIR)]
            xi_t = [[keep.tile([128, 512], BF, name=f"xi{p}_{e}")
                     for e in range(4)] for p in range(NPAIR)]
            xs_t = [[keep.tile([128, 512], BF, name=f"xs{p}_{e}")
                     for e in range(4)] for p in range(NPAIR)]
            kTr = [[keep.tile([128, 512], BF, name=f"kTr{p}_{u}")
                    for u in range(4)] for p in range(NPAIR)]
            kTi = [[keep.tile([128, 512], BF, name=f"kTi{p}_{u}")
                    for u in range(4)] for p in range(NPAIR)]
            vr = [[keep.tile([128, OUT], BF, name=f"vr{b}_{s}")
                   for s in range(2)] for b in range(BPC)]
            vi = [[keep.tile([128, OUT], BF, name=f"vi{b}_{s}")
                   for s in range(2)] for b in range(BPC)]
            w_sm = keep.tile([32, 2, S], BF)        # softmax weights (re|im)
            wexp = [keep.tile([NH, 2, 256], BF, name=f"wexp{b}")
                    for b in range(BPC)]            # per-batch exp staging
            den8 = [keep.tile([NH, 2], F32, name=f"den8_{b}") for b in range(BPC)]
            den = keep.tile([32, 2], F32)           # exp row-sums of cols 1..256
            e0 = keep.tile([32, 2], F32)            # exp of the s=0 logit
            den2 = keep.tile([32, 2], F32)
            rs = keep.tile([32, 2], F32)
            wTr = [keep.tile([128, 32], BF, name=f"wTr{a}") for a in range(2)]
            wTi = [keep.tile([128, 32], BF, name=f"wTi{a}") for a in range(2)]
            wTin = [keep.tile([128, 32], BF, name=f"wTin{a}") for a in range(2)]
            wt0a = keep.tile([2, 32], BF)           # rows (w0r, -w0i)
            wt0b = keep.tile([2, 32], BF)           # rows (w0r, w0i)
            wt0n = keep.tile([2, 32], BF)
            hvm_r = keep.tile([32, OUT], BF)
            hvm_i = keep.tile([32, OUT], BF)
            hvm_rb = [keep.tile([NH, OUT], BF, name=f"hvr{b}") for b in range(BPC)]
            hvm_ib = [keep.tile([NH, OUT], BF, name=f"hvi{b}") for b in range(BPC)]
            att_r = [keep.tile([128, BPC], BF, name=f"atr{u}") for u in range(4)]
            att_i = [keep.tile([128, BPC], BF, name=f"ati{u}") for u in range(4)]
            att_in = [keep.tile([128, BPC], BF, name=f"atn{u}") for u in range(4)]
            s12 = [keep.tile([128, 512], F32, name=f"s12_{j}") for j in range(2)]
            c2s = [keep.tile([128, 512], F32, name=f"c2_{j}") for j in range(2)]
            c1s = [keep.tile([128, 512], F32, name=f"c1_{j}") for j in range(2)]
            y_r = keep.tile([BPC, OUT], F32)
            y_i = keep.tile([BPC, OUT], F32)

            # ---- DMA emission; first-needed bytes first ----
            # k-weights split across sync+scalar queues so they land fastest
            for d_w, w_t in ((d_wr, w_r), (d_wi, w_i), (d_ws, w_s)):
                for e in range(4):
                    sl = slice(e * 128, (e + 1) * 128)
                    q = nc.sync if e < 2 else nc.scalar
                    q.dma_start(out=w_t[e][:, KS], in_=d_w.ap()[sl, KS])
            # x: one DMA per (pair, e, part) thanks to host pre-pairing.
            # pair0 on gpsimd (xr first), pair1 on scalar behind its k-weights
            for e in range(4):
                sl = slice(e * 128, (e + 1) * 128)
                nc.gpsimd.dma_start(out=xr_t[0][e][:], in_=d_xr.ap()[0, sl, :, :])
            for e in range(4):
                sl = slice(e * 128, (e + 1) * 128)
                nc.gpsimd.dma_start(out=xi_t[0][e][:], in_=d_xi.ap()[0, sl, :, :])
            for e in range(4):
                sl = slice(e * 128, (e + 1) * 128)
                nc.gpsimd.dma_start(out=xr_t[1][e][:], in_=d_xr.ap()[1, sl, :, :])
            for e in range(4):
                sl = slice(e * 128, (e + 1) * 128)
                nc.gpsimd.dma_start(out=xi_t[1][e][:], in_=d_xi.ap()[1, sl, :, :])
            # smalls + v-weights + wc stream in on sync under the k phase
            for u in range(4):
                sl = slice(u * 128, (u + 1) * 128)
                nc.sync.dma_start(out=bd_r[u], in_=d_bdr.ap()[sl, :])
                nc.sync.dma_start(out=bd_in[u], in_=d_bdin.ap()[sl, :])
                nc.sync.dma_start(out=bd_i[u], in_=d_bdi.ap()[sl, :])
            nc.sync.dma_start(out=lg0, in_=d_lg0.ap())
            nc.sync.dma_start(out=v02, in_=d_v02.ap())
            nc.sync.dma_start(out=v02s, in_=d_v02s.ap())
            nc.sync.dma_start(out=id32, in_=d_id32.ap())
            nc.sync.dma_start(out=mask8, in_=d_mask.ap())
            nc.sync.dma_start(out=sel32, in_=d_sel.ap())
            for d_w, w_t in ((d_wr, w_r), (d_wi, w_i), (d_ws, w_s)):
                for e in range(4):
                    sl = slice(e * 128, (e + 1) * 128)
                    nc.sync.dma_start(out=w_t[e][:, VS], in_=d_w.ap()[sl, VS])
            for e in range(4):
                sl = slice(e * 128, (e + 1) * 128)
                nc.sync.dma_start(out=wcr[e], in_=d_wcr.ap()[sl, :])
                nc.sync.dma_start(out=wci[e], in_=d_wci.ap()[sl, :])

            # Karatsuba sums of pair-0 x on vector; pair-1 emitted after the
            # pair-0 k combines so they don't block the vector queue
            for e in range(4):
                nc.vector.tensor_add(xs_t[0][e][:], xr_t[0][e][:],
                                     xi_t[0][e][:])

            stL = contextlib.ExitStack()
            psL = stL.enter_context(
                tc.tile_pool(name="psL", bufs=1, space="PSUM"))
            st = contextlib.ExitStack()
            psK = st.enter_context(
                tc.tile_pool(name="psK", bufs=1, space="PSUM"))

            nt = 0

            def k_uhalf(p, uh):
                nonlocal nt
                us = (2 * uh, 2 * uh + 1)
                tl = {}
                for kind, w_k, x_k in (("t1", w_r, xr_t), ("t2", w_i, xi_t),
                                       ("t3", w_s, xs_t)):
                    for u in us:
                        fs = slice(u * 128, (u + 1) * 128)
                        t = psK.tile([128, 512], F32, tag=kind, bufs=2,
                                     name=f"k{kind}_{p}_{u}")
                        tl[(kind, u)] = t
                        for e in range(4):
                            nc.tensor.matmul(t[:], w_k[e][:, fs], x_k[p][e][:],
                                             start=(e == 0), stop=(e == 3))
                for u in us:
                    t1, t2, t3 = tl[("t1", u)], tl[("t2", u)], tl[("t3", u)]
                    sc, c2 = s12[nt % 2], c2s[nt % 2]
                    nc.scalar.copy(c2[:], t2[:])
                    nc.vector.tensor_sub(kTr[p][u][:], t1[:], c2[:])
                    nc.vector.tensor_add(sc[:], t1[:], c2[:])
                    nc.vector.tensor_sub(kTi[p][u][:], t3[:], sc[:])
                    nt += 1

            def logits_batch(b):
                p, hf = divmod(b, 2)
                cs = slice(hf * 256, (hf + 1) * 256)
                bs = slice(b * 8, (b + 1) * 8)
                lr = psL.tile([8, 256], F32, tag="lr", name=f"lr{b}")
                li = psL.tile([8, 256], F32, tag="li", name=f"li{b}")
                for u in range(4):
                    nc.tensor.matmul(lr[:], bd_r[u][:, bs], kTr[p][u][:, cs],
                                     start=(u == 0), stop=False)
                    nc.tensor.matmul(lr[:], bd_in[u][:, bs], kTi[p][u][:, cs],
                                     start=False, stop=(u == 3))
                for u in range(4):
                    nc.tensor.matmul(li[:], bd_r[u][:, bs], kTi[p][u][:, cs],
                                     start=(u == 0), stop=False)
                    nc.tensor.matmul(li[:], bd_i[u][:, bs], kTr[p][u][:, cs],
                                     start=False, stop=(u == 3))
                nc.scalar.activation(wexp[b][:, 0, :], lr[:], ACTF.Exp,
                                     bias=0.0, scale=1.0,
                                     accum_out=den8[b][:, 0:1])
                nc.scalar.activation(wexp[b][:, 1, :], li[:], ACTF.Exp,
                                     bias=0.0, scale=1.0,
                                     accum_out=den8[b][:, 1:2])
                # engines can't write at partition offset b*8; DMA can
                nc.scalar.dma_start(out=w_sm[bs, :, 1:S], in_=wexp[b][:])
                nc.scalar.dma_start(out=den[bs, :], in_=den8[b][:])

            # ---- k + logits, interleaved so PE never waits on combines ----
            k_uhalf(0, 0)
            k_uhalf(0, 1)
            for e in range(4):
                nc.vector.tensor_add(xs_t[1][e][:], xr_t[1][e][:],
                                     xi_t[1][e][:])
            k_uhalf(1, 0)
            logits_batch(0)
            logits_batch(1)
            k_uhalf(1, 1)
            logits_batch(2)
            logits_batch(3)
            st.close()   # psK

            # ---- softmax tail: s=0 column + normalization ----
            nc.scalar.activation(e0[:], lg0[:], ACTF.Exp, bias=0.0, scale=1.0)
            nc.vector.tensor_copy(w_sm[:, :, 0], e0[:])
            nc.vector.tensor_add(den2[:], den[:], e0[:])
            nc.vector.reciprocal(rs[:], den2[:])
            nc.vector.tensor_scalar_mul(w_sm[:, 0, :], w_sm[:, 0, :], rs[:, 0:1])
            nc.vector.tensor_scalar_mul(w_sm[:, 1, :], w_sm[:, 1, :], rs[:, 1:2])
            stL.close()  # psL

            stV = contextlib.ExitStack()
            psV = stV.enter_context(
                tc.tile_pool(name="psV", bufs=1, space="PSUM"))

            def v_batch(b):
                nonlocal nt
                p, hf = divmod(b, 2)
                for stt in range(2):
                    scs = slice(hf * 256 + stt * 128,
                                hf * 256 + (stt + 1) * 128)
                    tl = {}
                    for kind, bufs, w_off, x_k in (
                            ("t1", 2, w_r, xr_t), ("t2", 2, w_i, xi_t),
                            ("t3", 2, w_s, xs_t)):
                        t = psV.tile([128, 512], F32, tag=kind, bufs=bufs,
                                     name=f"v{kind}_{b}_{stt}")
                        tl[kind] = t
                        for e in range(4):
                            nc.tensor.matmul(t[:], x_k[p][e][:, scs],
                                             w_off[e][:, VS],
                                             start=(e == 0), stop=(e == 3))
                    # offload the SBUF-only part of this combine to gpsimd
                    c1, c2, sc = c1s[nt % 2], c2s[nt % 2], s12[nt % 2]
                    nc.scalar.copy(c1[:], tl["t1"][:])
                    nc.scalar.copy(c2[:], tl["t2"][:])
                    nc.gpsimd.tensor_sub(vr[b][stt][:], c1[:], c2[:])
                    nc.gpsimd.tensor_add(sc[:], c1[:], c2[:])
                    nc.vector.tensor_sub(vi[b][stt][:], tl["t3"][:], sc[:])
                    nt += 1

            def hv_batch(b, psH):
                bs = slice(b * 8, (b + 1) * 8)
                ph_r = psH.tile([NH, OUT], F32, tag="hr", name=f"phr{b}")
                ph_i = psH.tile([NH, OUT], F32, tag="hi", name=f"phi{b}")
                mm = nc.tensor.matmul
                mm(ph_r[:], wTr[0][:, bs], vr[b][0][:], start=True, stop=False)
                mm(ph_r[:], wTr[1][:, bs], vr[b][1][:], start=False, stop=False)
                mm(ph_r[:], wTin[0][:, bs], vi[b][0][:], start=False, stop=False)
                mm(ph_r[:], wTin[1][:, bs], vi[b][1][:], start=False, stop=False)
                mm(ph_r[:], wt0a[:, bs], v02[:, b, :], start=False, stop=True)
                mm(ph_i[:], wTi[0][:, bs], vr[b][0][:], start=True, stop=False)
                mm(ph_i[:], wTi[1][:, bs], vr[b][1][:], start=False, stop=False)
                mm(ph_i[:], wTr[0][:, bs], vi[b][0][:], start=False, stop=False)
                mm(ph_i[:], wTr[1][:, bs], vi[b][1][:], start=False, stop=False)
                mm(ph_i[:], wt0b[:, bs], v02s[:, b, :], start=False, stop=True)
                nc.vector.tensor_mul(hvm_rb[b][:], ph_r[:], mask8[:])
                nc.vector.tensor_mul(hvm_ib[b][:], ph_i[:], mask8[:])
                nc.gpsimd.dma_start(out=hvm_r[bs, :], in_=hvm_rb[b][:])
                nc.gpsimd.dma_start(out=hvm_i[bs, :], in_=hvm_ib[b][:])

            v_batch(0)

            # ---- softmax-weight transposes -> [128s, 32bh] ----
            with tc.tile_pool(name="psT", bufs=1, space="PSUM") as psT:
                for a in range(2):
                    cs = slice(1 + a * 128, 1 + (a + 1) * 128)
                    ptr = psT.tile([128, 32], BF, tag="tw", bufs=2, name=f"ptr{a}")
                    pti = psT.tile([128, 32], BF, tag="tw", bufs=2, name=f"pti{a}")
                    nc.tensor.transpose(ptr[:], w_sm[:, 0, cs], id32[:])
                    nc.tensor.transpose(pti[:], w_sm[:, 1, cs], id32[:])
                    nc.scalar.copy(wTr[a][:], ptr[:])
                    nc.scalar.copy(wTi[a][:], pti[:])
                    nc.scalar.activation(wTin[a][:], pti[:], ACTF.Copy,
                                         bias=0.0, scale=-1.0)
                # s=0 row of both parts in one [32, 2] -> [2, 32] transpose
                ptc_t = psT.tile([128, 32], BF, tag="tw", bufs=2, name="ptc")
                ptc = ptc_t[0:2, :]
                nc.tensor.transpose(ptc[:], w_sm[:, :, 0], id32[:])
                nc.scalar.copy(wt0b[:], ptc[:])
                nc.scalar.activation(wt0n[:], ptc[:], ACTF.Copy,
                                     bias=0.0, scale=-1.0)
                nc.sync.dma_start(out=wt0a[0:1, :], in_=wt0b[0:1, :])
                nc.sync.dma_start(out=wt0a[1:2, :], in_=wt0n[1:2, :])

            stH = contextlib.ExitStack()
            psH = stH.enter_context(
                tc.tile_pool(name="psH", bufs=1, space="PSUM"))
            v_batch(1)
            hv_batch(0, psH)
            v_batch(2)
            hv_batch(1, psH)
            v_batch(3)
            hv_batch(2, psH)
            hv_batch(3, psH)
            stH.close()  # psH
            stV.close()  # psV

            with tc.tile_pool(name="psY", bufs=1, space="PSUM") as psY:
                # ---- extract attn0^T [128f, 4b] via selection matmul ----
                for u in range(4):
                    fs = slice(u * 128, (u + 1) * 128)
                    par = psY.tile([128, BPC], F32, tag="par", bufs=2, name=f"par{u}")
                    pai = psY.tile([128, BPC], F32, tag="pai", bufs=2, name=f"pai{u}")
                    nc.tensor.matmul(par[:], hvm_r[:, fs], sel32[:],
                                     start=True, stop=True)
                    nc.tensor.matmul(pai[:], hvm_i[:, fs], sel32[:],
                                     start=True, stop=True)
                    nc.scalar.copy(att_r[u][:], par[:])
                    nc.scalar.copy(att_i[u][:], pai[:])
                    nc.scalar.activation(att_in[u][:], pai[:], ACTF.Copy,
                                         bias=0.0, scale=-1.0)

                # ---- y = attn0 @ Wc^T ----
                py_r = psY.tile([BPC, OUT], F32, tag="pyr")
                py_i = psY.tile([BPC, OUT], F32, tag="pyi")
                for j, u in enumerate(range(4)):
                    nc.tensor.matmul(py_r[:], att_r[u][:], wcr[u][:],
                                     start=(j == 0), stop=False)
                    nc.tensor.matmul(py_r[:], att_in[u][:], wci[u][:],
                                     start=False, stop=(j == 3))
                    nc.tensor.matmul(py_i[:], att_r[u][:], wci[u][:],
                                     start=(j == 0), stop=False)
                    nc.tensor.matmul(py_i[:], att_i[u][:], wcr[u][:],
                                     start=False, stop=(j == 3))
                nc.scalar.copy(y_r[:], py_r[:])
                nc.scalar.copy(y_i[:], py_i[:])
                nc.sync.dma_start(out=d_yr.ap(), in_=y_r[:])
                nc.sync.dma_start(out=d_yi.ap(), in_=y_i[:])

    nc.compile()
    return nc


def _host_prep(inputs):
    """Host-side math + per-core in_maps."""
    f32 = np.float32
    xr = np.ascontiguousarray(inputs["x_real"], dtype=f32).reshape(B, E, HW)
    xi = np.ascontiguousarray(inputs["x_imag"], dtype=f32).reshape(B, E, HW)
    pos = np.asarray(inputs["pos_r"], f32) + 1j * np.asarray(inputs["pos_i"], f32)
    w_in = np.asarray(inputs["w_in_r"], f32) + 1j * np.asarray(inputs["w_in_i"], f32)
    b_in = np.asarray(inputs["b_in_r"], f32) + 1j * np.asarray(inputs["b_in_i"], f32)
    w_out = np.asarray(inputs["w_out_r"], f32) + 1j * np.asarray(inputs["w_out_i"], f32)
    b_out = np.asarray(inputs["b_out_r"], f32) + 1j * np.asarray(inputs["b_out_i"], f32)
    w_p = np.asarray(inputs["w_p_r"], f32) + 1j * np.asarray(inputs["w_p_i"], f32)
    b_p = np.asarray(inputs["b_p_r"], f32) + 1j * np.asarray(inputs["b_p_i"], f32)

    # ---- host math for the s=0 (mean) token ----
    x0 = (xr.mean(-1, dtype=np.float64) + 1j * xi.mean(-1, dtype=np.float64)
          ).astype(np.complex64) + pos[:, 0]                     # [B, E]
    qs = 1.0 / np.sqrt(HD)
    q0 = (x0 @ w_in[:E].T + b_in[:E]) * qs                       # [B, E]
    k0 = x0 @ w_in[E:2 * E].T                                    # [B, E]
    v0 = x0 @ w_in[2 * E:].T                                     # [B, E]
    lg0c = np.einsum("bhd,bhd->bh", q0.reshape(B, NH, HD),
                     k0.reshape(B, NH, HD))                      # [B, NH]

    wc = w_p @ w_out                                             # [OUT, E]
    # v-bias exits via sum(softmax)=1; out/proj biases are linear offsets.
    b_v = b_in[2 * E:]
    y_bias = ((1 + 1j) * b_v) @ wc.T + b_out @ w_p.T + b_p       # [OUT]

    # pos folded into the shipped x; pre-paired [pair, E, 2b, 256s]
    xr_s = (xr + pos.real[None, :, 1:S]).reshape(NCORES, NPAIR, 2, E, HW)
    xi_s = (xi + pos.imag[None, :, 1:S]).reshape(NCORES, NPAIR, 2, E, HW)
    xr_s = np.ascontiguousarray(xr_s.transpose(0, 1, 3, 2, 4)).astype(BF16)
    xi_s = np.ascontiguousarray(xi_s.transpose(0, 1, 3, 2, 4)).astype(BF16)

    bf = lambda a: np.ascontiguousarray(a, dtype=f32).astype(BF16)
    wkv = w_in[E:].T                                             # [E, 2E] complex
    shared = dict(
        wr=bf(wkv.real), wi=bf(wkv.imag), ws=bf(wkv.real + wkv.imag),
        wcr=bf(wc.real.T), wci=bf(wc.imag.T),
        id32=np.eye(32, dtype=f32).astype(BF16),
    )
    mask8 = np.zeros((NH, OUT), f32)
    for h in range(NH):
        mask8[h, h * HD:(h + 1) * HD] = 1.0
    sel32 = np.zeros((32, BPC), f32)
    for b in range(BPC):
        sel32[b * 8:(b + 1) * 8, b] = 1.0
    shared["mask8"] = mask8.astype(BF16)
    shared["sel32"] = sel32.astype(BF16)

    in_maps = []
    for c in range(NCORES):
        bsl = slice(c * BPC, (c + 1) * BPC)
        q0c, v0c, lg0c_c = q0[bsl], v0[bsl], lg0c[bsl]
        # block-diag bd [E, 32]: row f (grouped per u-tile), col b*8 + h(f)
        bdr = np.zeros((E, 32), f32)
        bdi = np.zeros((E, 32), f32)
        fidx = np.arange(E)
        for b in range(BPC):
            bdr[fidx, b * 8 + fidx // HD] = q0c[b].real
            bdi[fidx, b * 8 + fidx // HD] = q0c[b].imag
        lg0m = np.empty((32, 2), f32)
        lg0m[:, 0] = lg0c_c.real.reshape(-1)
        lg0m[:, 1] = lg0c_c.imag.reshape(-1)
        v02 = np.stack([v0c.real.astype(f32), v0c.imag.astype(f32)])  # [2,BPC,OUT]
        m = dict(shared)
        m["bdr"] = bdr.astype(BF16)
        m["bdi"] = bdi.astype(BF16)
        m["bdin"] = (-bdi).astype(BF16)
        m["lg0"] = lg0m
        m["v02"] = v02.astype(BF16)
        m["v02s"] = v02[::-1].copy().astype(BF16)
        m["xr"] = xr_s[c]
        m["xi"] = xi_s[c]
        in_maps.append(m)
    return in_maps, y_bias.astype(np.complex64)


def _run(inputs, trace=False, **kw):
    from concourse.bass_utils import run_bass_kernel_spmd
    if "nc" not in _cached:
        _cached["nc"] = _build()
    nc = _cached["nc"]
    in_maps, y_bias = _host_prep(inputs)
    res = run_bass_kernel_spmd(nc, in_maps, core_ids=list(range(NCORES)),
                               trace=trace, **kw)
    out = np.empty((B, OUT), np.complex64)
    for c in range(NCORES):
        out[c * BPC:(c + 1) * BPC] = (res.results[c]["yr"]
                                      + 1j * res.results[c]["yi"])
    out += y_bias[None, :]
    return out, res


def kernel(**inputs) -> np.ndarray:
    out, _ = _run(inputs)
    return out


# revision 21
# speedup vs baseline: 1.7254x; 1.1126x over previous
"""Complex AttentionPool2d on 8 trn2 NeuronCores, data-parallel over batch.

Contract: kernel(**inputs) takes the FULL inputs from setup_inputs() and
returns the FULL [32, 512] complex64 output.

v3 design: all matmuls in bf16 (tolerance is 2e-2), Karatsuba (3 real
matmuls) for the dominant k/v projections, and every small/serial piece of
math moved to the host:
  host: pos folded into the shipped x (x' = pixels + pos, pre-paired
        [pair, E, 2b, 256s]); x0 = mean(x)+pos0; q0 = (x0 Wq + b_q)/8;
        k0 = x0 Wk; v0 = x0 Wv; logit[s=0] = q0.k0; block-diag bd tiles
        from q0; v0 row-pairs; y-bias added to the final output on host.
  device (per core, 4 batches as 2 column-packed pairs):
        kT[f, (b,s)]   = Wk @ x'                 # Karatsuba, f-major
        v[(b,st)]      = x'^T @ Wv               # Karatsuba, s-major
        logits[8h,256] = bd^T @ kT per batch; col 0 from host
        w = softmax(re) + i softmax(im)          # exp straight from PSUM
        attn0 = w^T v (+ w0 x v0 row term)       # per batch [8, 512]
        y = attn0 @ (w_p w_out)^T                # via sel-extracted att^T

Engine rules honored: GPSIMD can't touch PSUM; vector ops read at most one
PSUM operand (stage via scalar-engine copies); compute engines can't write
at non-32-aligned partition offsets (assemble via DMA).

Math identities: k-bias dropped (softmax invariant); v-bias exits through
sum(w)=1 as a constant y-offset (host-added); q-bias folded into host q0.
"""
import contextlib
import numpy as np
import ml_dtypes

B, E, HW, S = 32, 512, 256, 257
NH, HD = 8, 64
OUT = 512
NCORES = 8
BPC = B // NCORES   # batches per core
NPAIR = BPC // 2    # column-packed batch pairs
BF16 = ml_dtypes.bfloat16

_cached = {}


def _build():
    import concourse.bacc as bacc
    import concourse.tile as tile
    import concourse.mybir as mybir

    F32 = mybir.dt.float32
    BF = mybir.dt.bfloat16
    ACTF = mybir.ActivationFunctionType

    nc = bacc.Bacc("TRN2", target_bir_lowering=False, debug=False)

    # ---- DRAM I/O ----
    F8 = mybir.dt.float8e4
    d_xr = nc.dram_tensor("xr", [NPAIR, E, 2, HW], BF, kind="ExternalInput")
    d_xi = nc.dram_tensor("xi", [NPAIR, E, 2, HW], BF, kind="ExternalInput")
    d_x8r = nc.dram_tensor("x8r", [NPAIR, 2, 128, 2, 512], F8, kind="ExternalInput")
    d_x8i = nc.dram_tensor("x8i", [NPAIR, 2, 128, 2, 512], F8, kind="ExternalInput")
    d_w8r = nc.dram_tensor("w8r", [2, 128, 2, 512], F8, kind="ExternalInput")
    d_w8i = nc.dram_tensor("w8i", [2, 128, 2, 512], F8, kind="ExternalInput")
    d_w8n = nc.dram_tensor("w8n", [2, 128, 2, 512], F8, kind="ExternalInput")
    d_wr = nc.dram_tensor("wr", [E, 2 * E], BF, kind="ExternalInput")
    d_wi = nc.dram_tensor("wi", [E, 2 * E], BF, kind="ExternalInput")
    d_ws = nc.dram_tensor("ws", [E, 2 * E], BF, kind="ExternalInput")
    d_wcr = nc.dram_tensor("wcr", [E, OUT], BF, kind="ExternalInput")
    d_wci = nc.dram_tensor("wci", [E, OUT], BF, kind="ExternalInput")
    d_bdr = nc.dram_tensor("bdr", [E, 32], BF, kind="ExternalInput")
    d_bdi = nc.dram_tensor("bdi", [E, 32], BF, kind="ExternalInput")
    d_bdin = nc.dram_tensor("bdin", [E, 32], BF, kind="ExternalInput")
    d_lg0 = nc.dram_tensor("lg0", [32, 2], F32, kind="ExternalInput")
    d_v02 = nc.dram_tensor("v02", [2, BPC, OUT], BF, kind="ExternalInput")
    d_v02s = nc.dram_tensor("v02s", [2, BPC, OUT], BF, kind="ExternalInput")
    d_id32 = nc.dram_tensor("id32", [32, 32], BF, kind="ExternalInput")
    d_mask = nc.dram_tensor("mask8", [NH, OUT], BF, kind="ExternalInput")
    d_sel = nc.dram_tensor("sel32", [32, BPC], BF, kind="ExternalInput")
    d_yr = nc.dram_tensor("yr", [BPC, OUT], F32, kind="ExternalOutput")
    d_yi = nc.dram_tensor("yi", [BPC, OUT], F32, kind="ExternalOutput")

    KS = slice(0, 512)       # k columns of the packed kv weight
    VS = slice(512, 1024)    # v columns

    with tile.TileContext(nc) as tc, \
         nc.allow_low_precision(reason="bf16 kernel; tolerance is 2e-2"):
        with tc.tile_pool(name="consts", bufs=1) as consts, \
             tc.tile_pool(name="keep", bufs=1) as keep:
            # ---- persistent weights / constants ----
            w_r = [consts.tile([128, 2 * E], BF, name=f"wr{e}") for e in range(4)]
            w_i = [consts.tile([128, 2 * E], BF, name=f"wi{e}") for e in range(4)]
            w_s = [consts.tile([128, 2 * E], BF, name=f"ws{e}") for e in range(4)]
            wcr = [consts.tile([128, OUT], BF, name=f"wcr{e}") for e in range(4)]
            wci = [consts.tile([128, OUT], BF, name=f"wci{e}") for e in range(4)]
            bd_r = [consts.tile([128, 32], BF, name=f"bdr{u}") for u in range(4)]
            bd_i = [consts.tile([128, 32], BF, name=f"bdi{u}") for u in range(4)]
            bd_in = [consts.tile([128, 32], BF, name=f"bdin{u}") for u in range(4)]
            lg0 = consts.tile([32, 2], F32)
            v02 = consts.tile([2, BPC, OUT], BF)
            v02s = consts.tile([2, BPC, OUT], BF)
            id32 = consts.tile([32, 32], BF)
            mask8 = consts.tile([NH, OUT], BF)
            sel32 = consts.tile([32, BPC], BF)

            x8 = {dt: [[consts.tile([128, 2, 512], F8, name=f"x8{dt}_{p}_{c}")
                        for c in range(2)] for p in range(NPAIR)]
                  for dt in "ri"}
            w8 = {dt: [consts.tile([128, 2, 512], F8, name=f"w8{dt}_{c}")
                       for c in range(2)] for dt in "rin"}

            # ---- persistent activations ----
            xr_t = [[keep.tile([128, 512], BF, name=f"xr{p}_{e}")
                     for e in range(4)] for p in range(NPAIR)]
            xi_t = [[keep.tile([128, 512], BF, name=f"xi{p}_{e}")
                     for e in range(4)] for p in range(NPAIR)]
            xs_t = [[keep.tile([128, 512], BF, name=f"xs{p}_{e}")
                     for e in range(4)] for p in range(NPAIR)]
            kTr = [[keep.tile([128, 512], BF, name=f"kTr{p}_{u}")
                    for u in range(4)] for p in range(NPAIR)]
            kTi = [[keep.tile([128, 512], BF, name=f"kTi{p}_{u}")
                    for u in range(4)] for p in range(NPAIR)]
            vr = [[keep.tile([128, OUT], BF, name=f"vr{b}_{s}")
                   for s in range(2)] for b in range(BPC)]
            vi = [[keep.tile([128, OUT], BF, name=f"vi{b}_{s}")
                   for s in range(2)] for b in range(BPC)]
            w_sm = keep.tile([32, 2, S], BF)        # softmax weights (re|im)
            wexp = [keep.tile([NH, 2, 256], BF, name=f"wexp{b}")
                    for b in range(BPC)]            # per-batch exp staging
            den8 = [keep.tile([NH, 2], F32, name=f"den8_{b}") for b in range(BPC)]
            den = keep.tile([32, 2], F32)           # exp row-sums of cols 1..256
            e0 = keep.tile([32, 2], F32)            # exp of the s=0 logit
            den2 = keep.tile([32, 2], F32)
            rs = keep.tile([32, 2], F32)
            wTr = [keep.tile([128, 32], BF, name=f"wTr{a}") for a in range(2)]
            wTi = [keep.tile([128, 32], BF, name=f"wTi{a}") for a in range(2)]
            wTin = [keep.tile([128, 32], BF, name=f"wTin{a}") for a in range(2)]
            wt0a = keep.tile([2, 32], BF)           # rows (w0r, -w0i)
            wt0b = keep.tile([2, 32], BF)           # rows (w0r, w0i)
            wt0n = keep.tile([2, 32], BF)
            hvm_r = keep.tile([32, OUT], BF)
            hvm_i = keep.tile([32, OUT], BF)
            hvm_rb = [keep.tile([NH, OUT], BF, name=f"hvr{b}") for b in range(BPC)]
            hvm_ib = [keep.tile([NH, OUT], BF, name=f"hvi{b}") for b in range(BPC)]
            att_r = [keep.tile([128, BPC], BF, name=f"atr{u}") for u in range(4)]
            att_i = [keep.tile([128, BPC], BF, name=f"ati{u}") for u in range(4)]
            att_in = [keep.tile([128, BPC], BF, name=f"atn{u}") for u in range(4)]
            s12 = [keep.tile([128, 512], F32, name=f"s12_{j}") for j in range(2)]
            c2s = [keep.tile([128, 512], F32, name=f"c2_{j}") for j in range(2)]
            c1s = [keep.tile([128, 512], F32, name=f"c1_{j}") for j in range(2)]
            y_r = keep.tile([BPC, OUT], F32)
            y_i = keep.tile([BPC, OUT], F32)

            # ---- DMA emission; first-needed bytes first ----
            # fp8 k-path: weights on sync/scalar, x8 on gpsimd
            for c in range(2):
                nc.sync.dma_start(out=w8["r"][c], in_=d_w8r.ap()[c])
                nc.scalar.dma_start(out=w8["i"][c], in_=d_w8i.ap()[c])
                nc.sync.dma_start(out=w8["n"][c], in_=d_w8n.ap()[c])
            for c in range(2):
                nc.gpsimd.dma_start(out=x8["r"][0][c], in_=d_x8r.ap()[0, c])
                nc.gpsimd.dma_start(out=x8["i"][0][c], in_=d_x8i.ap()[0, c])
            for e in range(4):
                sl = slice(e * 128, (e + 1) * 128)
                nc.gpsimd.dma_start(out=xr_t[0][e][:], in_=d_xr.ap()[0, sl, :, :])
            for c in range(2):
                nc.gpsimd.dma_start(out=x8["r"][1][c], in_=d_x8r.ap()[1, c])
                nc.gpsimd.dma_start(out=x8["i"][1][c], in_=d_x8i.ap()[1, c])
            for e in range(4):
                sl = slice(e * 128, (e + 1) * 128)
                nc.gpsimd.dma_start(out=xi_t[0][e][:], in_=d_xi.ap()[0, sl, :, :])
            for e in range(4):
                sl = slice(e * 128, (e + 1) * 128)
                nc.gpsimd.dma_start(out=xr_t[1][e][:], in_=d_xr.ap()[1, sl, :, :])
            for e in range(4):
                sl = slice(e * 128, (e + 1) * 128)
                nc.gpsimd.dma_start(out=xi_t[1][e][:], in_=d_xi.ap()[1, sl, :, :])
            # v-weights (r first), bd, rest of weights, smalls on sync
            for e in range(4):
                sl = slice(e * 128, (e + 1) * 128)
                nc.sync.dma_start(out=w_r[e][:, VS], in_=d_wr.ap()[sl, VS])
            for u in range(4):
                sl = slice(u * 128, (u + 1) * 128)
                nc.sync.dma_start(out=bd_r[u], in_=d_bdr.ap()[sl, :])
                nc.sync.dma_start(out=bd_in[u], in_=d_bdin.ap()[sl, :])
                nc.sync.dma_start(out=bd_i[u], in_=d_bdi.ap()[sl, :])
            nc.sync.dma_start(out=lg0, in_=d_lg0.ap())
            for d_w, w_t in ((d_wi, w_i), (d_ws, w_s)):
                for e in range(4):
                    sl = slice(e * 128, (e + 1) * 128)
                    nc.sync.dma_start(out=w_t[e][:, VS], in_=d_w.ap()[sl, VS])
            nc.sync.dma_start(out=v02, in_=d_v02.ap())
            nc.sync.dma_start(out=v02s, in_=d_v02s.ap())
            nc.sync.dma_start(out=id32, in_=d_id32.ap())
            nc.sync.dma_start(out=mask8, in_=d_mask.ap())
            nc.sync.dma_start(out=sel32, in_=d_sel.ap())
            for e in range(4):
                sl = slice(e * 128, (e + 1) * 128)
                nc.sync.dma_start(out=wcr[e], in_=d_wcr.ap()[sl, :])
                nc.sync.dma_start(out=wci[e], in_=d_wci.ap()[sl, :])


            stL = contextlib.ExitStack()
            psL = stL.enter_context(
                tc.tile_pool(name="psL", bufs=1, space="PSUM"))
            st = contextlib.ExitStack()
            psK = st.enter_context(
                tc.tile_pool(name="psK", bufs=1, space="PSUM"))

            nt = 0

            DR = mybir.MatmulPerfMode.DoubleRow

            def k_uhalf(p, uh):
                us = (2 * uh, 2 * uh + 1)
                for u in us:
                    fs = slice(u * 128, (u + 1) * 128)
                    tre = psK.tile([128, 512], F32, tag="t1", bufs=3,
                                   name=f"kre_{p}_{u}")
                    tim = psK.tile([128, 512], F32, tag="t2", bufs=3,
                                   name=f"kim_{p}_{u}")
                    for j, (wd, xd) in enumerate((("r", "r"), ("n", "i"))):
                        for c in range(2):
                            nc.tensor.matmul(tre[:], w8[wd][c][:, :, fs],
                                             x8[xd][p][c][:],
                                             start=(j == 0 and c == 0),
                                             stop=(j == 1 and c == 1),
                                             perf_mode=DR)
                    for j, (wd, xd) in enumerate((("i", "r"), ("r", "i"))):
                        for c in range(2):
                            nc.tensor.matmul(tim[:], w8[wd][c][:, :, fs],
                                             x8[xd][p][c][:],
                                             start=(j == 0 and c == 0),
                                             stop=(j == 1 and c == 1),
                                             perf_mode=DR)
                    nc.scalar.copy(kTr[p][u][:], tre[:])
                    nc.vector.tensor_copy(kTi[p][u][:], tim[:])

            def logits_batch(b):
                p, hf = divmod(b, 2)
                cs = slice(hf * 256, (hf + 1) * 256)
                bs = slice(b * 8, (b + 1) * 8)
                lr = psL.tile([8, 256], F32, tag="lr", name=f"lr{b}")
                li = psL.tile([8, 256], F32, tag="li", name=f"li{b}")
                for u in range(4):
                    nc.tensor.matmul(lr[:], bd_r[u][:, bs], kTr[p][u][:, cs],
                                     start=(u == 0), stop=False)
                    nc.tensor.matmul(lr[:], bd_in[u][:, bs], kTi[p][u][:, cs],
                                     start=False, stop=(u == 3))
                for u in range(4):
                    nc.tensor.matmul(li[:], bd_r[u][:, bs], kTi[p][u][:, cs],
                                     start=(u == 0), stop=False)
                    nc.tensor.matmul(li[:], bd_i[u][:, bs], kTr[p][u][:, cs],
                                     start=False, stop=(u == 3))
                nc.scalar.activation(wexp[b][:, 0, :], lr[:], ACTF.Exp,
                                     bias=0.0, scale=1.0,
                                     accum_out=den8[b][:, 0:1])
                nc.scalar.activation(wexp[b][:, 1, :], li[:], ACTF.Exp,
                                     bias=0.0, scale=1.0,
                                     accum_out=den8[b][:, 1:2])
                # engines can't write at partition offset b*8; DMA can
                nc.scalar.dma_start(out=w_sm[bs, :, 1:S], in_=wexp[b][:])
                nc.scalar.dma_start(out=den[bs, :], in_=den8[b][:])

            # ---- k + logits, interleaved so PE never waits on combines ----
            k_uhalf(0, 0)
            k_uhalf(0, 1)
            k_uhalf(1, 0)
            logits_batch(0)
            logits_batch(1)
            k_uhalf(1, 1)
            for p in range(NPAIR):
                for e in range(4):
                    nc.vector.tensor_add(xs_t[p][e][:], xr_t[p][e][:],
                                         xi_t[p][e][:])
            logits_batch(2)
            logits_batch(3)
            st.close()   # psK

            # ---- softmax tail: s=0 column + normalization ----
            nc.scalar.activation(e0[:], lg0[:], ACTF.Exp, bias=0.0, scale=1.0)
            nc.vector.tensor_copy(w_sm[:, :, 0], e0[:])
            nc.vector.tensor_add(den2[:], den[:], e0[:])
            nc.vector.reciprocal(rs[:], den2[:])
            nc.vector.tensor_scalar_mul(w_sm[:, 0, :], w_sm[:, 0, :], rs[:, 0:1])
            nc.vector.tensor_scalar_mul(w_sm[:, 1, :], w_sm[:, 1, :], rs[:, 1:2])
            stL.close()  # psL

            stV = contextlib.ExitStack()
            psV = stV.enter_context(
                tc.tile_pool(name="psV", bufs=1, space="PSUM"))

            def v_batch(b):
                nonlocal nt
                p, hf = divmod(b, 2)
                for stt in range(2):
                    scs = slice(hf * 256 + stt * 128,
                                hf * 256 + (stt + 1) * 128)
                    tl = {}
                    for kind, bufs, w_off, x_k in (
                            ("t1", 2, w_r, xr_t), ("t2", 2, w_i, xi_t),
                            ("t3", 2, w_s, xs_t)):
                        t = psV.tile([128, 512], F32, tag=kind, bufs=bufs,
                                     name=f"v{kind}_{b}_{stt}")
                        tl[kind] = t
                        for e in range(4):
                            nc.tensor.matmul(t[:], x_k[p][e][:, scs],
                                             w_off[e][:, VS],
                                             start=(e == 0), stop=(e == 3))
                    # offload the SBUF-only part of this combine to gpsimd
                    c1, c2, sc = c1s[nt % 2], c2s[nt % 2], s12[nt % 2]
                    nc.scalar.copy(c1[:], tl["t1"][:])
                    nc.scalar.copy(c2[:], tl["t2"][:])
                    nc.gpsimd.tensor_sub(vr[b][stt][:], c1[:], c2[:])
                    nc.gpsimd.tensor_add(sc[:], c1[:], c2[:])
                    nc.vector.tensor_sub(vi[b][stt][:], tl["t3"][:], sc[:])
                    nt += 1

            def hv_batch(b, psH):
                bs = slice(b * 8, (b + 1) * 8)
                ph_r = psH.tile([NH, OUT], F32, tag="hr", name=f"phr{b}")
                ph_i = psH.tile([NH, OUT], F32, tag="hi", name=f"phi{b}")
                mm = nc.tensor.matmul
                mm(ph_r[:], wTr[0][:, bs], vr[b][0][:], start=True, stop=False)
                mm(ph_r[:], wTr[1][:, bs], vr[b][1][:], start=False, stop=False)
                mm(ph_r[:], wTin[0][:, bs], vi[b][0][:], start=False, stop=False)
                mm(ph_r[:], wTin[1][:, bs], vi[b][1][:], start=False, stop=False)
                mm(ph_r[:], wt0a[:, bs], v02[:, b, :], start=False, stop=True)
                mm(ph_i[:], wTi[0][:, bs], vr[b][0][:], start=True, stop=False)
                mm(ph_i[:], wTi[1][:, bs], vr[b][1][:], start=False, stop=False)
                mm(ph_i[:], wTr[0][:, bs], vi[b][0][:], start=False, stop=False)
                mm(ph_i[:], wTr[1][:, bs], vi[b][1][:], start=False, stop=False)
                mm(ph_i[:], wt0b[:, bs], v02s[:, b, :], start=False, stop=True)
                nc.vector.tensor_mul(hvm_rb[b][:], ph_r[:], mask8[:])
                nc.vector.tensor_mul(hvm_ib[b][:], ph_i[:], mask8[:])
                nc.gpsimd.dma_start(out=hvm_r[bs, :], in_=hvm_rb[b][:])
                nc.gpsimd.dma_start(out=hvm_i[bs, :], in_=hvm_ib[b][:])

            v_batch(0)

            # ---- softmax-weight transposes -> [128s, 32bh] ----
            with tc.tile_pool(name="psT", bufs=1, space="PSUM") as psT:
                for a in range(2):
                    cs = slice(1 + a * 128, 1 + (a + 1) * 128)
                    ptr = psT.tile([128, 32], BF, tag="tw", bufs=2, name=f"ptr{a}")
                    pti = psT.tile([128, 32], BF, tag="tw", bufs=2, name=f"pti{a}")
                    nc.tensor.transpose(ptr[:], w_sm[:, 0, cs], id32[:])
                    nc.tensor.transpose(pti[:], w_sm[:, 1, cs], id32[:])
                    nc.scalar.copy(wTr[a][:], ptr[:])
                    nc.scalar.copy(wTi[a][:], pti[:])
                    nc.scalar.activation(wTin[a][:], pti[:], ACTF.Copy,
                                         bias=0.0, scale=-1.0)
                # s=0 row of both parts in one [32, 2] -> [2, 32] transpose
                ptc_t = psT.tile([128, 32], BF, tag="tw", bufs=2, name="ptc")
                ptc = ptc_t[0:2, :]
                nc.tensor.transpose(ptc[:], w_sm[:, :, 0], id32[:])
                nc.scalar.copy(wt0b[:], ptc[:])
                nc.scalar.activation(wt0n[:], ptc[:], ACTF.Copy,
                                     bias=0.0, scale=-1.0)
                nc.sync.dma_start(out=wt0a[0:1, :], in_=wt0b[0:1, :])
                nc.sync.dma_start(out=wt0a[1:2, :], in_=wt0n[1:2, :])

            stH = contextlib.ExitStack()
            psH = stH.enter_context(
                tc.tile_pool(name="psH", bufs=1, space="PSUM"))
            v_batch(1)
            hv_batch(0, psH)
            v_batch(2)
            hv_batch(1, psH)
            v_batch(3)
            hv_batch(2, psH)
            hv_batch(3, psH)
            stH.close()  # psH
            stV.close()  # psV

            with tc.tile_pool(name="psY", bufs=1, space="PSUM") as psY:
                # ---- extract attn0^T [128f, 4b] via selection matmul ----
                for u in range(4):
                    fs = slice(u * 128, (u + 1) * 128)
                    par = psY.tile([128, BPC], F32, tag="par", bufs=2, name=f"par{u}")
                    pai = psY.tile([128, BPC], F32, tag="pai", bufs=2, name=f"pai{u}")
                    nc.tensor.matmul(par[:], hvm_r[:, fs], sel32[:],
                                     start=True, stop=True)
                    nc.tensor.matmul(pai[:], hvm_i[:, fs], sel32[:],
                                     start=True, stop=True)
                    nc.scalar.copy(att_r[u][:], par[:])
                    nc.scalar.copy(att_i[u][:], pai[:])
                    nc.scalar.activation(att_in[u][:], pai[:], ACTF.Copy,
                                         bias=0.0, scale=-1.0)

                # ---- y = attn0 @ Wc^T ----
                py_r = psY.tile([BPC, OUT], F32, tag="pyr")
                py_i = psY.tile([BPC, OUT], F32, tag="pyi")
                for j, u in enumerate(range(4)):
                    nc.tensor.matmul(py_r[:], att_r[u][:], wcr[u][:],
                                     start=(j == 0), stop=False)
                    nc.tensor.matmul(py_r[:], att_in[u][:], wci[u][:],
                                     start=False, stop=(j == 3))
                    nc.tensor.matmul(py_i[:], att_r[u][:], wci[u][:],
                                     start=(j == 0), stop=False)
                    nc.tensor.matmul(py_i[:], att_i[u][:], wcr[u][:],
                                     start=False, stop=(j == 3))
                nc.scalar.copy(y_r[:], py_r[:])
                nc.scalar.copy(y_i[:], py_i[:])
                nc.sync.dma_start(out=d_yr.ap(), in_=y_r[:])
                nc.sync.dma_start(out=d_yi.ap(), in_=y_i[:])

    nc.compile()
    return nc


def _host_prep(inputs):
    """Host-side math + per-core in_maps."""
    f32 = np.float32
    xr = np.ascontiguousarray(inputs["x_real"], dtype=f32).reshape(B, E, HW)
    xi = np.ascontiguousarray(inputs["x_imag"], dtype=f32).reshape(B, E, HW)
    pos = np.asarray(inputs["pos_r"], f32) + 1j * np.asarray(inputs["pos_i"], f32)
    w_in = np.asarray(inputs["w_in_r"], f32) + 1j * np.asarray(inputs["w_in_i"], f32)
    b_in = np.asarray(inputs["b_in_r"], f32) + 1j * np.asarray(inputs["b_in_i"], f32)
    w_out = np.asarray(inputs["w_out_r"], f32) + 1j * np.asarray(inputs["w_out_i"], f32)
    b_out = np.asarray(inputs["b_out_r"], f32) + 1j * np.asarray(inputs["b_out_i"], f32)
    w_p = np.asarray(inputs["w_p_r"], f32) + 1j * np.asarray(inputs["w_p_i"], f32)
    b_p = np.asarray(inputs["b_p_r"], f32) + 1j * np.asarray(inputs["b_p_i"], f32)

    # ---- host math for the s=0 (mean) token ----
    x0 = (xr.mean(-1, dtype=np.float64) + 1j * xi.mean(-1, dtype=np.float64)
          ).astype(np.complex64) + pos[:, 0]                     # [B, E]
    qs = 1.0 / np.sqrt(HD)
    q0 = (x0 @ w_in[:E].T + b_in[:E]) * qs                       # [B, E]
    k0 = x0 @ w_in[E:2 * E].T                                    # [B, E]
    v0 = x0 @ w_in[2 * E:].T                                     # [B, E]
    lg0c = np.einsum("bhd,bhd->bh", q0.reshape(B, NH, HD),
                     k0.reshape(B, NH, HD))                      # [B, NH]

    wc = w_p @ w_out                                             # [OUT, E]
    # v-bias exits via sum(softmax)=1; out/proj biases are linear offsets.
    b_v = b_in[2 * E:]
    y_bias = ((1 + 1j) * b_v) @ wc.T + b_out @ w_p.T + b_p       # [OUT]

    # pos folded into the shipped x; pre-paired [pair, E, 2b, 256s]
    xr_f = (xr + pos.real[None, :, 1:S]).reshape(NCORES, NPAIR, 2, E, HW)
    xi_f = (xi + pos.imag[None, :, 1:S]).reshape(NCORES, NPAIR, 2, E, HW)
    xr_f = np.ascontiguousarray(xr_f.transpose(0, 1, 3, 2, 4))
    xi_f = np.ascontiguousarray(xi_f.transpose(0, 1, 3, 2, 4))  # [c,p,E,2,HW]
    xr_s = xr_f.astype(BF16)
    xi_s = xi_f.astype(BF16)
    # fp8 k-path copies: [core, pair, chunk-pair, 128, half, (2b x 256s)]
    FP8 = ml_dtypes.float8_e4m3

    def to8(xf):
        a = (xf * 8.0).reshape(NCORES, NPAIR, 2, 2, 128, 2 * HW)
        return np.ascontiguousarray(a.transpose(0, 1, 2, 4, 3, 5)).astype(FP8)

    x8r = to8(xr_f)
    x8i = to8(xi_f)

    bf = lambda a: np.ascontiguousarray(a, dtype=f32).astype(BF16)
    wkv = w_in[E:].T                                             # [E, 2E] complex

    def w_to8(wk):
        a = np.ascontiguousarray(wk * 32.0).reshape(2, 2, 128, 512)
        return np.ascontiguousarray(a.transpose(0, 2, 1, 3)).astype(FP8)

    wk_r, wk_i = wkv.real[:, :E], wkv.imag[:, :E]
    shared = dict(
        wr=bf(wkv.real), wi=bf(wkv.imag), ws=bf(wkv.real + wkv.imag),
        wcr=bf(wc.real.T), wci=bf(wc.imag.T),
        w8r=w_to8(wk_r), w8i=w_to8(wk_i), w8n=w_to8(-wk_i),
        id32=np.eye(32, dtype=f32).astype(BF16),
    )
    mask8 = np.zeros((NH, OUT), f32)
    for h in range(NH):
        mask8[h, h * HD:(h + 1) * HD] = 1.0
    sel32 = np.zeros((32, BPC), f32)
    for b in range(BPC):
        sel32[b * 8:(b + 1) * 8, b] = 1.0
    shared["mask8"] = mask8.astype(BF16)
    shared["sel32"] = sel32.astype(BF16)

    in_maps = []
    for c in range(NCORES):
        bsl = slice(c * BPC, (c + 1) * BPC)
        q0c, v0c, lg0c_c = q0[bsl], v0[bsl], lg0c[bsl]
        # block-diag bd [E, 32]: row f (grouped per u-tile), col b*8 + h(f)
        bdr = np.zeros((E, 32), f32)
        bdi = np.zeros((E, 32), f32)
        fidx = np.arange(E)
        for b in range(BPC):
            bdr[fidx, b * 8 + fidx // HD] = q0c[b].real / 256.0
            bdi[fidx, b * 8 + fidx // HD] = q0c[b].imag / 256.0
        lg0m = np.empty((32, 2), f32)
        lg0m[:, 0] = lg0c_c.real.reshape(-1)
        lg0m[:, 1] = lg0c_c.imag.reshape(-1)
        v02 = np.stack([v0c.real.astype(f32), v0c.imag.astype(f32)])  # [2,BPC,OUT]
        m = dict(shared)
        m["bdr"] = bdr.astype(BF16)
        m["bdi"] = bdi.astype(BF16)
        m["bdin"] = (-bdi).astype(BF16)
        m["lg0"] = lg0m
        m["v02"] = v02.astype(BF16)
        m["v02s"] = v02[::-1].copy().astype(BF16)
        m["xr"] = xr_s[c]
        m["xi"] = xi_s[c]
        m["x8r"] = x8r[c]
        m["x8i"] = x8i[c]
        in_maps.append(m)
    return in_maps, y_bias.astype(np.complex64)


def _run(inputs, trace=False, **kw):
    from concourse.bass_utils import run_bass_kernel_spmd
    if "nc" not in _cached:
        _cached["nc"] = _build()
    nc = _cached["nc"]
    in_maps, y_bias = _host_prep(inputs)
    res = run_bass_kernel_spmd(nc, in_maps, core_ids=list(range(NCORES)),
                               trace=trace, **kw)
    out = np.empty((B, OUT), np.complex64)
    for c in range(NCORES):
        out[c * BPC:(c + 1) * BPC] = (res.results[c]["yr"]
                                      + 1j * res.results[c]["yi"])
    out += y_bias[None, :]
    return out, res


def kernel(**inputs) -> np.ndarray:
    out, _ = _run(inputs)
    return out


# revision 22
# speedup vs baseline: 1.7903x; 1.0376x over previous
"""Complex AttentionPool2d on 8 trn2 NeuronCores, data-parallel over batch.

Contract: kernel(**inputs) takes the FULL inputs from setup_inputs() and
returns the FULL [32, 512] complex64 output.

v3 design: all matmuls in bf16 (tolerance is 2e-2), Karatsuba (3 real
matmuls) for the dominant k/v projections, and every small/serial piece of
math moved to the host:
  host: pos folded into the shipped x (x' = pixels + pos, pre-paired
        [pair, E, 2b, 256s]); x0 = mean(x)+pos0; q0 = (x0 Wq + b_q)/8;
        k0 = x0 Wk; v0 = x0 Wv; logit[s=0] = q0.k0; block-diag bd tiles
        from q0; v0 row-pairs; y-bias added to the final output on host.
  device (per core, 4 batches as 2 column-packed pairs):
        kT[f, (b,s)]   = Wk @ x'                 # Karatsuba, f-major
        v[(b,st)]      = x'^T @ Wv               # Karatsuba, s-major
        logits[8h,256] = bd^T @ kT per batch; col 0 from host
        w = softmax(re) + i softmax(im)          # exp straight from PSUM
        attn0 = w^T v (+ w0 x v0 row term)       # per batch [8, 512]
        y = attn0 @ (w_p w_out)^T                # via sel-extracted att^T

Engine rules honored: GPSIMD can't touch PSUM; vector ops read at most one
PSUM operand (stage via scalar-engine copies); compute engines can't write
at non-32-aligned partition offsets (assemble via DMA).

Math identities: k-bias dropped (softmax invariant); v-bias exits through
sum(w)=1 as a constant y-offset (host-added); q-bias folded into host q0.
"""
import contextlib
import numpy as np
import ml_dtypes

B, E, HW, S = 32, 512, 256, 257
NH, HD = 8, 64
OUT = 512
NCORES = 8
BPC = B // NCORES   # batches per core
NPAIR = BPC // 2    # column-packed batch pairs
BF16 = ml_dtypes.bfloat16

_cached = {}


def _build():
    import concourse.bacc as bacc
    import concourse.tile as tile
    import concourse.mybir as mybir

    F32 = mybir.dt.float32
    BF = mybir.dt.bfloat16
    ACTF = mybir.ActivationFunctionType

    nc = bacc.Bacc("TRN2", target_bir_lowering=False, debug=False)

    # ---- DRAM I/O ----
    F8 = mybir.dt.float8e4
    d_xr = nc.dram_tensor("xr", [NPAIR, E, 2, HW], BF, kind="ExternalInput")
    d_xi = nc.dram_tensor("xi", [NPAIR, E, 2, HW], BF, kind="ExternalInput")
    d_x8r = nc.dram_tensor("x8r", [NPAIR, 2, 128, 2, 512], F8, kind="ExternalInput")
    d_x8i = nc.dram_tensor("x8i", [NPAIR, 2, 128, 2, 512], F8, kind="ExternalInput")
    d_w8r = nc.dram_tensor("w8r", [2, 128, 2, 512], F8, kind="ExternalInput")
    d_w8i = nc.dram_tensor("w8i", [2, 128, 2, 512], F8, kind="ExternalInput")
    d_w8n = nc.dram_tensor("w8n", [2, 128, 2, 512], F8, kind="ExternalInput")
    d_wr = nc.dram_tensor("wr", [E, 2 * E], BF, kind="ExternalInput")
    d_wi = nc.dram_tensor("wi", [E, 2 * E], BF, kind="ExternalInput")
    d_ws = nc.dram_tensor("ws", [E, 2 * E], BF, kind="ExternalInput")
    d_wcr = nc.dram_tensor("wcr", [E, OUT], BF, kind="ExternalInput")
    d_wci = nc.dram_tensor("wci", [E, OUT], BF, kind="ExternalInput")
    d_bdr = nc.dram_tensor("bdr", [E, 32], BF, kind="ExternalInput")
    d_bdi = nc.dram_tensor("bdi", [E, 32], BF, kind="ExternalInput")
    d_bdin = nc.dram_tensor("bdin", [E, 32], BF, kind="ExternalInput")
    d_lg0 = nc.dram_tensor("lg0", [32, 2], F32, kind="ExternalInput")
    d_v02 = nc.dram_tensor("v02", [2, BPC, OUT], BF, kind="ExternalInput")
    d_v02s = nc.dram_tensor("v02s", [2, BPC, OUT], BF, kind="ExternalInput")
    d_id32 = nc.dram_tensor("id32", [32, 32], BF, kind="ExternalInput")
    d_mask = nc.dram_tensor("mask8", [NH, OUT], BF, kind="ExternalInput")
    d_sel = nc.dram_tensor("sel32", [32, BPC], BF, kind="ExternalInput")
    d_yr = nc.dram_tensor("yr", [BPC, OUT], F32, kind="ExternalOutput")
    d_yi = nc.dram_tensor("yi", [BPC, OUT], F32, kind="ExternalOutput")

    KS = slice(0, 512)       # k columns of the packed kv weight
    VS = slice(512, 1024)    # v columns

    with tile.TileContext(nc) as tc, \
         nc.allow_low_precision(reason="bf16 kernel; tolerance is 2e-2"):
        with tc.tile_pool(name="consts", bufs=1) as consts, \
             tc.tile_pool(name="keep", bufs=1) as keep:
            # ---- persistent weights / constants ----
            w_r = [consts.tile([128, 2 * E], BF, name=f"wr{e}") for e in range(4)]
            w_i = [consts.tile([128, 2 * E], BF, name=f"wi{e}") for e in range(4)]
            w_s = [consts.tile([128, 2 * E], BF, name=f"ws{e}") for e in range(4)]
            wcr = [consts.tile([128, OUT], BF, name=f"wcr{e}") for e in range(4)]
            wci = [consts.tile([128, OUT], BF, name=f"wci{e}") for e in range(4)]
            bd_r = [consts.tile([128, 32], BF, name=f"bdr{u}") for u in range(4)]
            bd_i = [consts.tile([128, 32], BF, name=f"bdi{u}") for u in range(4)]
            bd_in = [consts.tile([128, 32], BF, name=f"bdin{u}") for u in range(4)]
            lg0 = consts.tile([32, 2], F32)
            v02 = consts.tile([2, BPC, OUT], BF)
            v02s = consts.tile([2, BPC, OUT], BF)
            id32 = consts.tile([32, 32], BF)
            mask8 = consts.tile([NH, OUT], BF)
            sel32 = consts.tile([32, BPC], BF)

            x8 = {dt: [[consts.tile([128, 2, 512], F8, name=f"x8{dt}_{p}_{c}")
                        for c in range(2)] for p in range(NPAIR)]
                  for dt in "ri"}
            w8 = {dt: [consts.tile([128, 2, 512], F8, name=f"w8{dt}_{c}")
                       for c in range(2)] for dt in "rin"}

            # ---- persistent activations ----
            xr_t = [[keep.tile([128, 512], BF, name=f"xr{p}_{e}")
                     for e in range(4)] for p in range(NPAIR)]
            xi_t = [[keep.tile([128, 512], BF, name=f"xi{p}_{e}")
                     for e in range(4)] for p in range(NPAIR)]
            xs_t = [[keep.tile([128, 512], BF, name=f"xs{p}_{e}")
                     for e in range(4)] for p in range(NPAIR)]
            kTr = [[keep.tile([128, 512], BF, name=f"kTr{p}_{u}")
                    for u in range(4)] for p in range(NPAIR)]
            kTi = [[keep.tile([128, 512], BF, name=f"kTi{p}_{u}")
                    for u in range(4)] for p in range(NPAIR)]
            vr = [[keep.tile([128, OUT], BF, name=f"vr{b}_{s}")
                   for s in range(2)] for b in range(BPC)]
            vi = [[keep.tile([128, OUT], BF, name=f"vi{b}_{s}")
                   for s in range(2)] for b in range(BPC)]
            w_sm = keep.tile([32, 2, S], BF)        # softmax weights (re|im)
            wexp = [keep.tile([NH, 2, 256], BF, name=f"wexp{b}")
                    for b in range(BPC)]            # per-batch exp staging
            den8 = [keep.tile([NH, 2], F32, name=f"den8_{b}") for b in range(BPC)]
            den = keep.tile([32, 2], F32)           # exp row-sums of cols 1..256
            e0 = keep.tile([32, 2], F32)            # exp of the s=0 logit
            den2 = keep.tile([32, 2], F32)
            rs = keep.tile([32, 2], F32)
            wTr = [keep.tile([128, 32], BF, name=f"wTr{a}") for a in range(2)]
            wTi = [keep.tile([128, 32], BF, name=f"wTi{a}") for a in range(2)]
            wTin = [keep.tile([128, 32], BF, name=f"wTin{a}") for a in range(2)]
            wt0a = keep.tile([2, 32], BF)           # rows (w0r, -w0i)
            wt0b = keep.tile([2, 32], BF)           # rows (w0r, w0i)
            wt0n = keep.tile([2, 32], BF)
            hvm_r = keep.tile([32, OUT], BF)
            hvm_i = keep.tile([32, OUT], BF)
            hvm_rb = [keep.tile([NH, OUT], BF, name=f"hvr{b}") for b in range(BPC)]
            hvm_ib = [keep.tile([NH, OUT], BF, name=f"hvi{b}") for b in range(BPC)]
            att_r = [keep.tile([128, BPC], BF, name=f"atr{u}") for u in range(4)]
            att_i = [keep.tile([128, BPC], BF, name=f"ati{u}") for u in range(4)]
            att_in = [keep.tile([128, BPC], BF, name=f"atn{u}") for u in range(4)]
            s12 = [keep.tile([128, 512], F32, name=f"s12_{j}") for j in range(2)]
            c2s = [keep.tile([128, 512], F32, name=f"c2_{j}") for j in range(2)]
            c1s = [keep.tile([128, 512], F32, name=f"c1_{j}") for j in range(2)]
            y_r = keep.tile([BPC, OUT], F32)
            y_i = keep.tile([BPC, OUT], F32)

            # ---- DMA emission; first-needed bytes first ----
            # fp8 k-path: weights on sync/scalar, x8 on gpsimd
            for c in range(2):
                nc.sync.dma_start(out=w8["r"][c], in_=d_w8r.ap()[c])
                nc.scalar.dma_start(out=w8["i"][c], in_=d_w8i.ap()[c])
                nc.sync.dma_start(out=w8["n"][c], in_=d_w8n.ap()[c])
            for c in range(2):
                nc.gpsimd.dma_start(out=x8["r"][0][c], in_=d_x8r.ap()[0, c])
                nc.gpsimd.dma_start(out=x8["i"][0][c], in_=d_x8i.ap()[0, c])
            for e in range(4):
                sl = slice(e * 128, (e + 1) * 128)
                nc.gpsimd.dma_start(out=xr_t[0][e][:], in_=d_xr.ap()[0, sl, :, :])
            for c in range(2):
                nc.gpsimd.dma_start(out=x8["r"][1][c], in_=d_x8r.ap()[1, c])
                nc.gpsimd.dma_start(out=x8["i"][1][c], in_=d_x8i.ap()[1, c])
            for e in range(4):
                sl = slice(e * 128, (e + 1) * 128)
                nc.gpsimd.dma_start(out=xi_t[0][e][:], in_=d_xi.ap()[0, sl, :, :])
            for e in range(4):
                sl = slice(e * 128, (e + 1) * 128)
                nc.gpsimd.dma_start(out=xr_t[1][e][:], in_=d_xr.ap()[1, sl, :, :])
            for e in range(4):
                sl = slice(e * 128, (e + 1) * 128)
                nc.gpsimd.dma_start(out=xi_t[1][e][:], in_=d_xi.ap()[1, sl, :, :])
            # v-weights (r first), bd, rest of weights, smalls on sync
            for e in range(4):
                sl = slice(e * 128, (e + 1) * 128)
                nc.sync.dma_start(out=w_r[e][:, VS], in_=d_wr.ap()[sl, VS])
            for u in range(4):
                sl = slice(u * 128, (u + 1) * 128)
                nc.sync.dma_start(out=bd_r[u], in_=d_bdr.ap()[sl, :])
                nc.sync.dma_start(out=bd_in[u], in_=d_bdin.ap()[sl, :])
                nc.sync.dma_start(out=bd_i[u], in_=d_bdi.ap()[sl, :])
            nc.sync.dma_start(out=lg0, in_=d_lg0.ap())
            for d_w, w_t in ((d_wi, w_i), (d_ws, w_s)):
                for e in range(4):
                    sl = slice(e * 128, (e + 1) * 128)
                    nc.sync.dma_start(out=w_t[e][:, VS], in_=d_w.ap()[sl, VS])
            nc.sync.dma_start(out=v02, in_=d_v02.ap())
            nc.sync.dma_start(out=v02s, in_=d_v02s.ap())
            nc.sync.dma_start(out=id32, in_=d_id32.ap())
            nc.sync.dma_start(out=mask8, in_=d_mask.ap())
            nc.sync.dma_start(out=sel32, in_=d_sel.ap())
            for e in range(4):
                sl = slice(e * 128, (e + 1) * 128)
                nc.sync.dma_start(out=wcr[e], in_=d_wcr.ap()[sl, :])
                nc.sync.dma_start(out=wci[e], in_=d_wci.ap()[sl, :])


            # PE p-state warm-up: ~3us of dummy matmuls while DMAs land
            dummy = keep.tile([128, 512], BF)
            nc.gpsimd.memset(dummy[:], 0.0)
            with tc.tile_pool(name="psW", bufs=1, space="PSUM") as psW:
                pw = psW.tile([128, 128], F32, tag="w", bufs=2)
                for j in range(14):
                    nc.tensor.matmul(pw[:], dummy[:, 0:128], dummy[:, 0:128],
                                     start=True, stop=True)

            stL = contextlib.ExitStack()
            psL = stL.enter_context(
                tc.tile_pool(name="psL", bufs=1, space="PSUM"))
            st = contextlib.ExitStack()
            psK = st.enter_context(
                tc.tile_pool(name="psK", bufs=1, space="PSUM"))

            nt = 0

            DR = mybir.MatmulPerfMode.DoubleRow

            def k_uhalf(p, uh):
                us = (2 * uh, 2 * uh + 1)
                for u in us:
                    fs = slice(u * 128, (u + 1) * 128)
                    tre = psK.tile([128, 512], F32, tag="t1", bufs=3,
                                   name=f"kre_{p}_{u}")
                    tim = psK.tile([128, 512], F32, tag="t2", bufs=3,
                                   name=f"kim_{p}_{u}")
                    for j, (wd, xd) in enumerate((("r", "r"), ("n", "i"))):
                        for c in range(2):
                            nc.tensor.matmul(tre[:], w8[wd][c][:, :, fs],
                                             x8[xd][p][c][:],
                                             start=(j == 0 and c == 0),
                                             stop=(j == 1 and c == 1),
                                             perf_mode=DR)
                    for j, (wd, xd) in enumerate((("i", "r"), ("r", "i"))):
                        for c in range(2):
                            nc.tensor.matmul(tim[:], w8[wd][c][:, :, fs],
                                             x8[xd][p][c][:],
                                             start=(j == 0 and c == 0),
                                             stop=(j == 1 and c == 1),
                                             perf_mode=DR)
                    nc.scalar.copy(kTr[p][u][:], tre[:])
                    nc.vector.tensor_copy(kTi[p][u][:], tim[:])

            def logits_batch(b):
                p, hf = divmod(b, 2)
                cs = slice(hf * 256, (hf + 1) * 256)
                bs = slice(b * 8, (b + 1) * 8)
                lr = psL.tile([8, 256], F32, tag="lr", name=f"lr{b}")
                li = psL.tile([8, 256], F32, tag="li", name=f"li{b}")
                for u in range(4):
                    nc.tensor.matmul(lr[:], bd_r[u][:, bs], kTr[p][u][:, cs],
                                     start=(u == 0), stop=False)
                    nc.tensor.matmul(lr[:], bd_in[u][:, bs], kTi[p][u][:, cs],
                                     start=False, stop=(u == 3))
                for u in range(4):
                    nc.tensor.matmul(li[:], bd_r[u][:, bs], kTi[p][u][:, cs],
                                     start=(u == 0), stop=False)
                    nc.tensor.matmul(li[:], bd_i[u][:, bs], kTr[p][u][:, cs],
                                     start=False, stop=(u == 3))
                nc.scalar.activation(wexp[b][:, 0, :], lr[:], ACTF.Exp,
                                     bias=0.0, scale=1.0,
                                     accum_out=den8[b][:, 0:1])
                nc.scalar.activation(wexp[b][:, 1, :], li[:], ACTF.Exp,
                                     bias=0.0, scale=1.0,
                                     accum_out=den8[b][:, 1:2])
                # engines can't write at partition offset b*8; DMA can
                nc.scalar.dma_start(out=w_sm[bs, :, 1:S], in_=wexp[b][:])
                nc.scalar.dma_start(out=den[bs, :], in_=den8[b][:])

            # ---- k + logits, interleaved so PE never waits on combines ----
            k_uhalf(0, 0)
            k_uhalf(0, 1)
            k_uhalf(1, 0)
            logits_batch(0)
            logits_batch(1)
            k_uhalf(1, 1)
            for p in range(NPAIR):
                for e in range(4):
                    nc.vector.tensor_add(xs_t[p][e][:], xr_t[p][e][:],
                                         xi_t[p][e][:])
            logits_batch(2)
            logits_batch(3)
            st.close()   # psK

            # ---- softmax tail: s=0 column + normalization ----
            nc.scalar.activation(e0[:], lg0[:], ACTF.Exp, bias=0.0, scale=1.0)
            nc.vector.tensor_copy(w_sm[:, :, 0], e0[:])
            nc.vector.tensor_add(den2[:], den[:], e0[:])
            nc.vector.reciprocal(rs[:], den2[:])
            nc.vector.tensor_scalar_mul(w_sm[:, 0, :], w_sm[:, 0, :], rs[:, 0:1])
            nc.vector.tensor_scalar_mul(w_sm[:, 1, :], w_sm[:, 1, :], rs[:, 1:2])
            stL.close()  # psL

            stV = contextlib.ExitStack()
            psV = stV.enter_context(
                tc.tile_pool(name="psV", bufs=1, space="PSUM"))

            def v_batch(b):
                nonlocal nt
                p, hf = divmod(b, 2)
                for stt in range(2):
                    scs = slice(hf * 256 + stt * 128,
                                hf * 256 + (stt + 1) * 128)
                    tl = {}
                    for kind, bufs, w_off, x_k in (
                            ("t1", 2, w_r, xr_t), ("t2", 2, w_i, xi_t),
                            ("t3", 2, w_s, xs_t)):
                        t = psV.tile([128, 512], F32, tag=kind, bufs=bufs,
                                     name=f"v{kind}_{b}_{stt}")
                        tl[kind] = t
                        for e in range(4):
                            nc.tensor.matmul(t[:], x_k[p][e][:, scs],
                                             w_off[e][:, VS],
                                             start=(e == 0), stop=(e == 3))
                    # offload the SBUF-only part of this combine to gpsimd
                    c1, c2, sc = c1s[nt % 2], c2s[nt % 2], s12[nt % 2]
                    nc.scalar.copy(c1[:], tl["t1"][:])
                    nc.scalar.copy(c2[:], tl["t2"][:])
                    nc.gpsimd.tensor_sub(vr[b][stt][:], c1[:], c2[:])
                    nc.gpsimd.tensor_add(sc[:], c1[:], c2[:])
                    nc.vector.tensor_sub(vi[b][stt][:], tl["t3"][:], sc[:])
                    nt += 1

            def hv_batch(b, psH):
                bs = slice(b * 8, (b + 1) * 8)
                ph_r = psH.tile([NH, OUT], F32, tag="hr", name=f"phr{b}")
                ph_i = psH.tile([NH, OUT], F32, tag="hi", name=f"phi{b}")
                mm = nc.tensor.matmul
                mm(ph_r[:], wTr[0][:, bs], vr[b][0][:], start=True, stop=False)
                mm(ph_r[:], wTr[1][:, bs], vr[b][1][:], start=False, stop=False)
                mm(ph_r[:], wTin[0][:, bs], vi[b][0][:], start=False, stop=False)
                mm(ph_r[:], wTin[1][:, bs], vi[b][1][:], start=False, stop=False)
                mm(ph_r[:], wt0a[:, bs], v02[:, b, :], start=False, stop=True)
                mm(ph_i[:], wTi[0][:, bs], vr[b][0][:], start=True, stop=False)
                mm(ph_i[:], wTi[1][:, bs], vr[b][1][:], start=False, stop=False)
                mm(ph_i[:], wTr[0][:, bs], vi[b][0][:], start=False, stop=False)
                mm(ph_i[:], wTr[1][:, bs], vi[b][1][:], start=False, stop=False)
                mm(ph_i[:], wt0b[:, bs], v02s[:, b, :], start=False, stop=True)
                nc.vector.tensor_mul(hvm_rb[b][:], ph_r[:], mask8[:])
                nc.vector.tensor_mul(hvm_ib[b][:], ph_i[:], mask8[:])
                nc.gpsimd.dma_start(out=hvm_r[bs, :], in_=hvm_rb[b][:])
                nc.gpsimd.dma_start(out=hvm_i[bs, :], in_=hvm_ib[b][:])

            v_batch(0)

            # ---- softmax-weight transposes -> [128s, 32bh] ----
            with tc.tile_pool(name="psT", bufs=1, space="PSUM") as psT:
                for a in range(2):
                    cs = slice(1 + a * 128, 1 + (a + 1) * 128)
                    ptr = psT.tile([128, 32], BF, tag="tw", bufs=2, name=f"ptr{a}")
                    pti = psT.tile([128, 32], BF, tag="tw", bufs=2, name=f"pti{a}")
                    nc.tensor.transpose(ptr[:], w_sm[:, 0, cs], id32[:])
                    nc.tensor.transpose(pti[:], w_sm[:, 1, cs], id32[:])
                    nc.scalar.copy(wTr[a][:], ptr[:])
                    nc.scalar.copy(wTi[a][:], pti[:])
                    nc.scalar.activation(wTin[a][:], pti[:], ACTF.Copy,
                                         bias=0.0, scale=-1.0)
                # s=0 row of both parts in one [32, 2] -> [2, 32] transpose
                ptc_t = psT.tile([128, 32], BF, tag="tw", bufs=2, name="ptc")
                ptc = ptc_t[0:2, :]
                nc.tensor.transpose(ptc[:], w_sm[:, :, 0], id32[:])
                nc.scalar.copy(wt0b[:], ptc[:])
                nc.scalar.activation(wt0n[:], ptc[:], ACTF.Copy,
                                     bias=0.0, scale=-1.0)
                nc.sync.dma_start(out=wt0a[0:1, :], in_=wt0b[0:1, :])
                nc.sync.dma_start(out=wt0a[1:2, :], in_=wt0n[1:2, :])

            stH = contextlib.ExitStack()
            psH = stH.enter_context(
                tc.tile_pool(name="psH", bufs=1, space="PSUM"))
            v_batch(1)
            hv_batch(0, psH)
            v_batch(2)
            hv_batch(1, psH)
            v_batch(3)
            hv_batch(2, psH)
            hv_batch(3, psH)
            stH.close()  # psH
            stV.close()  # psV

            with tc.tile_pool(name="psY", bufs=1, space="PSUM") as psY:
                # ---- extract attn0^T [128f, 4b] via selection matmul ----
                for u in range(4):
                    fs = slice(u * 128, (u + 1) * 128)
                    par = psY.tile([128, BPC], F32, tag="par", bufs=2, name=f"par{u}")
                    pai = psY.tile([128, BPC], F32, tag="pai", bufs=2, name=f"pai{u}")
                    nc.tensor.matmul(par[:], hvm_r[:, fs], sel32[:],
                                     start=True, stop=True)
                    nc.tensor.matmul(pai[:], hvm_i[:, fs], sel32[:],
                                     start=True, stop=True)
                    nc.scalar.copy(att_r[u][:], par[:])
                    nc.scalar.copy(att_i[u][:], pai[:])
                    nc.scalar.activation(att_in[u][:], pai[:], ACTF.Copy,
                                         bias=0.0, scale=-1.0)

                # ---- y = attn0 @ Wc^T ----
                py_r = psY.tile([BPC, OUT], F32, tag="pyr")
                py_i = psY.tile([BPC, OUT], F32, tag="pyi")
                for j, u in enumerate(range(4)):
                    nc.tensor.matmul(py_r[:], att_r[u][:], wcr[u][:],
                                     start=(j == 0), stop=False)
                    nc.tensor.matmul(py_r[:], att_in[u][:], wci[u][:],
                                     start=False, stop=(j == 3))
                    nc.tensor.matmul(py_i[:], att_r[u][:], wci[u][:],
                                     start=(j == 0), stop=False)
                    nc.tensor.matmul(py_i[:], att_i[u][:], wcr[u][:],
                                     start=False, stop=(j == 3))
                nc.scalar.copy(y_r[:], py_r[:])
                nc.vector.tensor_copy(y_i[:], py_i[:])
                nc.sync.dma_start(out=d_yr.ap(), in_=y_r[:])
                nc.scalar.dma_start(out=d_yi.ap(), in_=y_i[:])

    nc.compile()
    return nc


def _host_prep(inputs):
    """Host-side math + per-core in_maps."""
    f32 = np.float32
    xr = np.ascontiguousarray(inputs["x_real"], dtype=f32).reshape(B, E, HW)
    xi = np.ascontiguousarray(inputs["x_imag"], dtype=f32).reshape(B, E, HW)
    pos = np.asarray(inputs["pos_r"], f32) + 1j * np.asarray(inputs["pos_i"], f32)
    w_in = np.asarray(inputs["w_in_r"], f32) + 1j * np.asarray(inputs["w_in_i"], f32)
    b_in = np.asarray(inputs["b_in_r"], f32) + 1j * np.asarray(inputs["b_in_i"], f32)
    w_out = np.asarray(inputs["w_out_r"], f32) + 1j * np.asarray(inputs["w_out_i"], f32)
    b_out = np.asarray(inputs["b_out_r"], f32) + 1j * np.asarray(inputs["b_out_i"], f32)
    w_p = np.asarray(inputs["w_p_r"], f32) + 1j * np.asarray(inputs["w_p_i"], f32)
    b_p = np.asarray(inputs["b_p_r"], f32) + 1j * np.asarray(inputs["b_p_i"], f32)

    # ---- host math for the s=0 (mean) token ----
    x0 = (xr.mean(-1, dtype=np.float64) + 1j * xi.mean(-1, dtype=np.float64)
          ).astype(np.complex64) + pos[:, 0]                     # [B, E]
    qs = 1.0 / np.sqrt(HD)
    q0 = (x0 @ w_in[:E].T + b_in[:E]) * qs                       # [B, E]
    k0 = x0 @ w_in[E:2 * E].T                                    # [B, E]
    v0 = x0 @ w_in[2 * E:].T                                     # [B, E]
    lg0c = np.einsum("bhd,bhd->bh", q0.reshape(B, NH, HD),
                     k0.reshape(B, NH, HD))                      # [B, NH]

    wc = w_p @ w_out                                             # [OUT, E]
    # v-bias exits via sum(softmax)=1; out/proj biases are linear offsets.
    b_v = b_in[2 * E:]
    y_bias = ((1 + 1j) * b_v) @ wc.T + b_out @ w_p.T + b_p       # [OUT]

    # pos folded into the shipped x; pre-paired [pair, E, 2b, 256s]
    xr_f = (xr + pos.real[None, :, 1:S]).reshape(NCORES, NPAIR, 2, E, HW)
    xi_f = (xi + pos.imag[None, :, 1:S]).reshape(NCORES, NPAIR, 2, E, HW)
    xr_f = np.ascontiguousarray(xr_f.transpose(0, 1, 3, 2, 4))
    xi_f = np.ascontiguousarray(xi_f.transpose(0, 1, 3, 2, 4))  # [c,p,E,2,HW]
    xr_s = xr_f.astype(BF16)
    xi_s = xi_f.astype(BF16)
    # fp8 k-path copies: [core, pair, chunk-pair, 128, half, (2b x 256s)]
    FP8 = ml_dtypes.float8_e4m3

    def to8(xf):
        a = (xf * 8.0).reshape(NCORES, NPAIR, 2, 2, 128, 2 * HW)
        return np.ascontiguousarray(a.transpose(0, 1, 2, 4, 3, 5)).astype(FP8)

    x8r = to8(xr_f)
    x8i = to8(xi_f)

    bf = lambda a: np.ascontiguousarray(a, dtype=f32).astype(BF16)
    wkv = w_in[E:].T                                             # [E, 2E] complex

    def w_to8(wk):
        a = np.ascontiguousarray(wk * 32.0).reshape(2, 2, 128, 512)
        return np.ascontiguousarray(a.transpose(0, 2, 1, 3)).astype(FP8)

    wk_r, wk_i = wkv.real[:, :E], wkv.imag[:, :E]
    shared = dict(
        wr=bf(wkv.real), wi=bf(wkv.imag), ws=bf(wkv.real + wkv.imag),
        wcr=bf(wc.real.T), wci=bf(wc.imag.T),
        w8r=w_to8(wk_r), w8i=w_to8(wk_i), w8n=w_to8(-wk_i),
        id32=np.eye(32, dtype=f32).astype(BF16),
    )
    mask8 = np.zeros((NH, OUT), f32)
    for h in range(NH):
        mask8[h, h * HD:(h + 1) * HD] = 1.0
    sel32 = np.zeros((32, BPC), f32)
    for b in range(BPC):
        sel32[b * 8:(b + 1) * 8, b] = 1.0
    shared["mask8"] = mask8.astype(BF16)
    shared["sel32"] = sel32.astype(BF16)

    in_maps = []
    for c in range(NCORES):
        bsl = slice(c * BPC, (c + 1) * BPC)
        q0c, v0c, lg0c_c = q0[bsl], v0[bsl], lg0c[bsl]
        # block-diag bd [E, 32]: row f (grouped per u-tile), col b*8 + h(f)
        bdr = np.zeros((E, 32), f32)
        bdi = np.zeros((E, 32), f32)
        fidx = np.arange(E)
        for b in range(BPC):
            bdr[fidx, b * 8 + fidx // HD] = q0c[b].real / 256.0
            bdi[fidx, b * 8 + fidx // HD] = q0c[b].imag / 256.0
        lg0m = np.empty((32, 2), f32)
        lg0m[:, 0] = lg0c_c.real.reshape(-1)
        lg0m[:, 1] = lg0c_c.imag.reshape(-1)
        v02 = np.stack([v0c.real.astype(f32), v0c.imag.astype(f32)])  # [2,BPC,OUT]
        m = dict(shared)
        m["bdr"] = bdr.astype(BF16)
        m["bdi"] = bdi.astype(BF16)
        m["bdin"] = (-bdi).astype(BF16)
        m["lg0"] = lg0m
        m["v02"] = v02.astype(BF16)
        m["v02s"] = v02[::-1].copy().astype(BF16)
        m["xr"] = xr_s[c]
        m["xi"] = xi_s[c]
        m["x8r"] = x8r[c]
        m["x8i"] = x8i[c]
        in_maps.append(m)
    return in_maps, y_bias.astype(np.complex64)


def _run(inputs, trace=False, **kw):
    from concourse.bass_utils import run_bass_kernel_spmd
    if "nc" not in _cached:
        _cached["nc"] = _build()
    nc = _cached["nc"]
    in_maps, y_bias = _host_prep(inputs)
    res = run_bass_kernel_spmd(nc, in_maps, core_ids=list(range(NCORES)),
                               trace=trace, **kw)
    out = np.empty((B, OUT), np.complex64)
    for c in range(NCORES):
        out[c * BPC:(c + 1) * BPC] = (res.results[c]["yr"]
                                      + 1j * res.results[c]["yi"])
    out += y_bias[None, :]
    return out, res


def kernel(**inputs) -> np.ndarray:
    out, _ = _run(inputs)
    return out
